# revision 17
# baseline (speedup 1.0000x reference)
"""Trainium2 Bass kernel for a 2-layer relational GNN (EvalNet).

Strategy (v2): shard by destination node with a *balanced* node->
(core,tile,slot) assignment (equalizes per-(core,tile,relation) edge
counts, minimizing gather padding). Layer-1 aggregations are core-local.

Layer 1 per dst tile: gather x[src] rows (bf16) per edge; reconstruct
x[dst] per edge ON-CHIP as a one-hot matmul against the tile's own x
(eliminating the dst gather entirely); edge weight via DVE
scalar_tensor_tensor with row-accumulate; weighted one-hot scatter into
per-relation PSUM; apply rel_W after aggregation.

x1 is AllGathered in fp8 (nag=2 slabs, overlapped with L1). Layer 2
gathers fp8 x1 rows per edge (src-deduped per (tile,slab) with
multi-hot one-hots streamed from host), scatters with oh2 as the
stationary matmul operand, transposes the aggregate on-chip, and
applies mp_lin/mp_self after aggregation. The own-node (self) path uses
the locally kept bf16 x1 (no gather). pooled is reduced on-chip;
the final 5-way projection of the 768-d pooled mean is on host.
"""

import os
import sys

sys.path.insert(0, "/opt/trn_rl_repo")

import numpy as np
import ml_dtypes

import concourse.bacc as bacc
import concourse.tile as tile
import concourse.mybir as mybir
from concourse import bass_utils
from concourse.library_config import mlp as mlp_lib

BF16 = ml_dtypes.bfloat16
F8 = ml_dtypes.float8_e4m3

N = 16384
E = 262144
R = 9
DIN = 384
H = 768
NCLS = 5
NCORES = 8
NOWN = N // NCORES          # 2048 nodes per core
NT = NOWN // 128            # 16 dst tiles of 128 per core
NBIN = NCORES * NT          # 128 (core,tile) bins
NAG = 2                     # AllGather slabs
X1SCALE = 0.125             # x1 -> fp8 scale (undone in lint host-side)

FP32 = mybir.dt.float32
BF = mybir.dt.bfloat16
F8E4 = mybir.dt.float8e4
I16 = mybir.dt.int16
AX = mybir.AxisListType
ALU = mybir.AluOpType
ACTF = mybir.ActivationFunctionType


def _wrap16(ids):
    """int16 index layout for dma_gather: [128, n/16], element i at
    [i%16 (+16r replicas), i//16]."""
    a = np.asarray(ids, np.int16).reshape(-1, 16).T  # [16, n/16]
    return np.ascontiguousarray(np.tile(a, (8, 1)))


def _roundup(x, m):
    return (x + m - 1) // m * m


def _balance_bins(dst, et):
    """Assign nodes to NBIN bins of 128 slots, balancing the 9-dim
    in-degree vectors. Returns pos[node] = global position (bin*128+slot)."""
    deg = np.zeros((N, R), np.int64)
    np.add.at(deg, (dst, et), 1)
    order = np.argsort(-deg.sum(1), kind="stable")
    bin_cnt = np.zeros((NBIN, R), np.float64)
    bin_n = np.zeros(NBIN, np.int64)
    pos = np.zeros(N, np.int64)
    for n in order:
        d = deg[n].astype(np.float64)
        # min over open bins of dot(current load, node's degree vector)
        cost = bin_cnt @ d + 1e-6 * bin_cnt.sum(1)
        cost[bin_n >= 128] = np.inf
        b = int(np.argmin(cost))
        pos[n] = b * 128 + bin_n[b]
        bin_n[b] += 1
        bin_cnt[b] += d
    assert (bin_n == 128).all()
    return pos


def _ag_row(pos):
    """x1 table row (after slab-major AllGather) for global position pos."""
    c, loc = pos // NOWN, pos % NOWN
    g, l = loc // (NOWN // NAG), loc % (NOWN // NAG)
    return g * (NCORES * (NOWN // NAG)) + c * (NOWN // NAG) + l


def _prep(src, dst, et, invnc):
    pos = _balance_bins(dst, et)
    dcore = pos[dst] // NOWN
    dtile = (pos[dst] % NOWN) // 128
    dslot = pos[dst] % 128
    # source AG slab (for L2 gather gating): slab of the src node's tile
    sslab = (pos[src] % NOWN) // (NOWN // NAG)

    per_core = [np.nonzero(dcore == c)[0] for c in range(NCORES)]

    # ---------- L1: group by (tile, rel) ----------
    counts1 = np.zeros((NCORES, NT, R), np.int64)
    for c in range(NCORES):
        e = per_core[c]
        np.add.at(counts1[c], (dtile[e], et[e]), 1)
    K1 = _roundup(counts1.max(axis=0), 128)     # [NT, R]
    S1 = K1.sum(axis=1)                          # [NT]
    E1 = int(S1.sum())
    NCH1 = E1 // 128

    sched1 = []
    for t in range(NT):
        gs, c0 = [], 0
        for r in range(R):
            nch = int(K1[t, r]) // 128
            if nch:
                gs.append((r, c0, c0 + nch))
                c0 += nch
        sched1.append(gs)

    # ---------- L2: group by (tile, slab), dedup by src ----------
    # distinct-src counts per (core,tile,slab)
    counts2 = np.zeros((NCORES, NT, NAG), np.int64)
    groups2 = {}
    for c in range(NCORES):
        e = per_core[c]
        for t in range(NT):
            sel_t = e[dtile[e] == t]
            for g in range(NAG):
                es = sel_t[sslab[sel_t] == g]
                srcs, inv_idx = np.unique(src[es], return_inverse=True)
                counts2[c, t, g] = srcs.size
                groups2[(c, t, g)] = (srcs, inv_idx, es)
    K2 = _roundup(np.maximum(counts2.max(axis=0), 1), 128)  # [NT, NAG]
    S2 = K2.sum(axis=1)
    E2 = int(S2.sum())
    NCH2 = E2 // 128

    sched2 = []   # per tile: [(g, chunk0, chunk1)]
    for t in range(NT):
        gs, c0 = [], 0
        for g in range(NAG):
            nch = int(K2[t, g]) // 128
            gs.append((g, c0, c0 + nch))
            c0 += nch
        sched2.append(gs)

    cores = []
    for c in range(NCORES):
        src1 = np.zeros(E1, np.int64)
        slot1 = np.full(E1, -1.0, np.float32)
        inv1 = np.zeros(E1, np.float32)
        rel1 = np.full(E1, -1, np.int64)
        off = 0
        e_all = per_core[c]
        for t in range(NT):
            for r in range(R):
                k = int(K1[t, r])
                if k == 0:
                    continue
                es = e_all[(dtile[e_all] == t) & (et[e_all] == r)]
                n = es.size
                src1[off:off + n] = src[es]
                slot1[off:off + n] = dslot[es]
                inv1[off:off + n] = invnc[es]
                rel1[off:off + n] = r
                off += k
        assert off == E1

        # ohuT1[s, e] one-hot of slot (fp8), rn1[e%128, chunk*R + r]
        ohuT1 = np.zeros((128, E1), F8)
        val = slot1 >= 0
        idx = np.nonzero(val)[0]
        ohuT1[slot1[idx].astype(np.int64), idx] = inv1[idx]
        rn1 = np.zeros((128, NCH1 * R), BF16)
        rn1[idx % 128, (idx // 128) * R + rel1[idx]] = 1.0

        src2 = np.zeros(E2, np.int64)
        oh2 = np.zeros((128, E2), np.float32)
        off = 0
        for t in range(NT):
            for g in range(NAG):
                k = int(K2[t, g])
                srcs, inv_idx, es = groups2[(c, t, g)]
                n = srcs.size
                src2[off:off + n] = _ag_row(pos[srcs])
                np.add.at(oh2, (dslot[es], off + inv_idx), 1.0)
                off += k
        assert off == E2
        oh2 = oh2.astype(F8)
        # reshape oh2 to [128 p=e%128, chunk*128 + s]
        oh2v = np.zeros((128, NCH2 * 128), F8)
        er = np.arange(E2)
        oh2v[:, :] = oh2.T.reshape(NCH2, 128, 128).transpose(1, 0, 2).reshape(128, -1)

        def wrap128(v):
            o = np.zeros((128, v.size // 128), np.float32)
            p = np.arange(v.size)
            o[p % 128, p // 128] = v
            return o

        own = np.argsort(pos)[c * NOWN:(c + 1) * NOWN]  # node ids in (t,s) order
        deg2 = np.zeros(NOWN, np.float32)
        lp = pos[dst[e_all]] - c * NOWN
        np.add.at(deg2, lp, 1.0)
        degones = np.stack([deg2, np.ones(NOWN, np.float32)])

        cores.append(dict(
            src1=_wrap16(src1), slot1=wrap128(slot1),
            rn1=rn1, ohuT1=ohuT1,
            src2=_wrap16(src2), oh2=oh2v,
            degones=degones, own=own,
        ))

    return dict(E1=E1, NCH1=NCH1, S1=S1, sched1=sched1,
                E2=E2, NCH2=NCH2, S2=S2, sched2=sched2,
                K2m=int(K2.max()) // 128, cores=cores, pos=pos)


def _build(sch):
    E1, NCH1, S1, sched1 = sch["E1"], sch["NCH1"], sch["S1"], sch["sched1"]
    E2, NCH2, S2, sched2 = sch["E2"], sch["NCH2"], sch["S2"], sch["sched2"]
    G1 = max(int(s) for s in S1) // 128
    G2 = max(int(s) for s in S2) // 128
    K2m = sch["K2m"]
    gblk = int(os.environ.get("GBLK", 9))
    gblk2 = int(os.environ.get("GBLK2", 8))
    scratch = int(os.environ.get("DMA_SCRATCH", 16384))

    nc = bacc.Bacc("TRN2", target_bir_lowering=False, debug=False,
                   num_devices=NCORES, dynamic_dma_scratch_size=scratch)

    xb_d = nc.dram_tensor("xb", [N, DIN], BF, kind="ExternalInput")
    xown_d = nc.dram_tensor("xown", [128, NT * DIN], F8E4, kind="ExternalInput")
    relwt_d = nc.dram_tensor("relwt", [R, DIN, H], BF, kind="ExternalInput")
    relb_d = nc.dram_tensor("relb", [R, H], FP32, kind="ExternalInput")
    lint_d = nc.dram_tensor("lint", [H, H], BF, kind="ExternalInput")
    selft_d = nc.dram_tensor("selft", [H, H], BF, kind="ExternalInput")
    b2_d = nc.dram_tensor("b2", [2, H], FP32, kind="ExternalInput")
    degones_d = nc.dram_tensor("degones", [2, NOWN], FP32, kind="ExternalInput")
    src1_d = nc.dram_tensor("src1", [128, E1 // 16], I16, kind="ExternalInput")
    slot1_d = nc.dram_tensor("slot1", [128, NCH1], FP32, kind="ExternalInput")
    rn1_d = nc.dram_tensor("rn1", [128, NCH1 * R], BF, kind="ExternalInput")
    ohuT1_d = nc.dram_tensor("ohuT1", [128, E1], F8E4, kind="ExternalInput")
    src2_d = nc.dram_tensor("src2", [128, E2 // 16], I16, kind="ExternalInput")
    oh2_d = nc.dram_tensor("oh2", [128, NCH2 * 128], F8E4, kind="ExternalInput")
    iota_d = nc.dram_tensor("iota", [128, 128], FP32, kind="ExternalInput")
    ident_d = nc.dram_tensor("ident", [128, 128], BF, kind="ExternalInput")
    pooled_d = nc.dram_tensor("pooled", [128, 6], FP32, kind="ExternalOutput")

    with tile.TileContext(nc) as tc:
        nc.gpsimd.load_library(mlp_lib)
        with (
            tc.tile_pool(name="const", bufs=1) as cp,
            tc.tile_pool(name="dram", bufs=1, space="DRAM") as dp,
        ):
            # ---- metadata / small constants (gate first gathers) ----
            src1_sb = cp.tile([128, E1 // 16], I16)
            nc.sync.dma_start(src1_sb[:], src1_d[:])
            slot1_sb = cp.tile([128, NCH1], FP32)
            nc.sync.dma_start(slot1_sb[:], slot1_d[:])
            rn1_sb = cp.tile([128, NCH1 * R], BF)
            nc.sync.dma_start(rn1_sb[:], rn1_d[:])
            iota_sb = cp.tile([128, 128], FP32)
            nc.sync.dma_start(iota_sb[:], iota_d[:])
            ident_sb = cp.tile([128, 128], BF)
            nc.sync.dma_start(ident_sb[:], ident_d[:])
            src2_sb = cp.tile([128, E2 // 16], I16)
            nc.sync.dma_start(src2_sb[:], src2_d[:])
            xown_sb = cp.tile([128, NT * DIN], F8E4)
            nc.sync.dma_start(xown_sb[:], xown_d[:])
            relb_sb = cp.tile([R, H], FP32)
            nc.sync.dma_start(relb_sb[:], relb_d[:])
            b2_sb = cp.tile([2, H], FP32)
            nc.sync.dma_start(b2_sb[:], b2_d[:])
            degones_sb = cp.tile([2, NOWN], FP32)
            nc.sync.dma_start(degones_sb[:], degones_d[:])
            lint_sb = cp.tile([128, 6 * H], BF)
            selft_sb = cp.tile([128, 6 * H], BF)
            nc.sync.dma_start(
                lint_sb.rearrange("p (k h) -> p k h", h=H)[:, :, :],
                lint_d.rearrange("(k p) h -> p k h", p=128)[:, :, :])
            nc.sync.dma_start(
                selft_sb.rearrange("p (k h) -> p k h", h=H)[:, :, :],
                selft_d.rearrange("(k p) h -> p k h", p=128)[:, :, :])
            pooled_sb = cp.tile([128, 6], FP32)
            nc.vector.memset(pooled_sb[:], 0.0)
            # x1 kept locally (bf16) + transposed-on-demand for L2 self path
            x1own_sb = cp.tile([128, NT * H], BF)

            cc_in = dp.tile([NOWN, H], F8E4)
            cc_out = dp.tile([N, H], F8E4)

            def sub_gather(dst_tile, src_ap, idx_sb, chunk0, nchunks, elem,
                           blk, out_chunk0=0):
                v3 = dst_tile.rearrange("p (c d) -> p c d", d=elem)
                for b0 in range(0, nchunks, blk):
                    b1 = min(b0 + blk, nchunks)
                    col = (chunk0 + b0) * 8
                    nc.gpsimd.dma_gather(
                        v3[:, out_chunk0 + b0:out_chunk0 + b1, :], src_ap,
                        idx_sb[:, col:col + (b1 - b0) * 8],
                        (b1 - b0) * 128, (b1 - b0) * 128, elem,
                        single_packet=False)

            # ================= Layer 1 =================
            with (
                tc.tile_pool(name="w1c", bufs=1) as wc,
                tc.tile_pool(name="w1", bufs=2) as wp,
                tc.tile_pool(name="ps1", bufs=2, space="PSUM") as pp,
            ):
                relwt_sb = wc.tile([128, R * 3 * H], BF)
                nc.sync.dma_start(
                    relwt_sb.rearrange("p (r k h) -> p r k h", k=3, h=H)[:, :, :, :],
                    relwt_d.rearrange("r (k p) h -> p r k h", p=128)[:, :, :, :])

                chunk_base = 0
                for t in range(NT):
                    ncht = int(S1[t]) // 128
                    xs_g = wp.tile([128, G1 * DIN], BF, tag="xs")
                    sub_gather(xs_g, xb_d[:], src1_sb, chunk_base, ncht, DIN,
                               gblk)
                    ohuT_t = wp.tile([128, G1 * 128], F8E4, tag="ohuT")
                    nc.sync.dma_start(
                        ohuT_t[:, :ncht * 128],
                        ohuT1_d[:, chunk_base * 128:(chunk_base + ncht) * 128])

                    art_sb = wp.tile([128, R * 3 * 128], BF, tag="artsb")
                    ct_ps = pp.tile([R, 128], FP32, tag="ct", bufs=1)

                    first_ct = True
                    for (r, gc0, gc1) in sched1[t]:
                        art_ps = pp.tile([128, 3 * 128], FP32, tag="art",
                                         bufs=2)
                        for ci in range(gc0, gc1):
                            gci = chunk_base + ci
                            xs_c = xs_g[:, ci * DIN:(ci + 1) * DIN]
                            # reconstruct x_dst rows for this chunk on PE
                            xd_ps = pp.tile([128, DIN], FP32, tag="xd",
                                            bufs=2)
                            nc.tensor.matmul(
                                xd_ps[:],
                                ohuT_t[:, ci * 128:(ci + 1) * 128],
                                xown_sb[:, t * DIN:(t + 1) * DIN],
                                start=True, stop=True)
                            norm = wp.tile([128, 1], FP32, tag="norm", bufs=4)
                            prod = wp.tile([128, DIN], BF, tag="prod", bufs=4)
                            nc.vector.tensor_mul(prod[:], xs_c, xd_ps[:])
                            dmy = wp.tile([128, DIN], BF, tag="dmy", bufs=4)
                            nc.scalar.activation(dmy[:], prod[:], ACTF.Copy,
                                                 accum_out=norm[:])
                            ohw = wp.tile([128, 128], BF, tag="ohw", bufs=4)
                            nc.vector.tensor_scalar(
                                ohw[:], iota_sb[:], slot1_sb[:, gci:gci + 1],
                                norm[:], ALU.is_equal, ALU.mult)
                            nc.tensor.matmul(
                                ct_ps[:], rn1_sb[:, gci * R:(gci + 1) * R],
                                ohw[:], start=first_ct,
                                stop=(ci == sched1[t][-1][2] - 1))
                            first_ct = False
                            for k in range(3):
                                nc.tensor.matmul(
                                    art_ps[:, k * 128:(k + 1) * 128],
                                    xs_c[:, k * 128:(k + 1) * 128],
                                    ohw[:], start=(ci == gc0 and k == 0),
                                    stop=(ci == gc1 - 1 and k == 2))
                        nc.scalar.copy(
                            art_sb[:, r * 384:(r + 1) * 384], art_ps[:])

                    ct_sb = wp.tile([R, 128], FP32, tag="ctsb")
                    nc.vector.tensor_copy(ct_sb[:], ct_ps[:])

                    x1t = wp.tile([128, H], BF, tag="x1t")
                    for s in range(2):
                        mps = pp.tile([128, 384], FP32, tag="mps", bufs=2)
                        first = True
                        for (r, _, _) in sched1[t]:
                            for k in range(3):
                                nc.tensor.matmul(
                                    mps[:],
                                    art_sb[:, r * 384 + k * 128:
                                           r * 384 + (k + 1) * 128],
                                    relwt_sb[:, (r * 3 + k) * H + s * 384:
                                             (r * 3 + k) * H + (s + 1) * 384],
                                    start=first, stop=False)
                                first = False
                        nc.tensor.matmul(mps[:], ct_sb[:],
                                         relb_sb[:, s * 384:(s + 1) * 384],
                                         start=False, stop=True)
                        nc.scalar.activation(x1t[:, s * 384:(s + 1) * 384],
                                             mps[:], ACTF.Relu)
                    nc.vector.tensor_copy(x1own_sb[:, t * H:(t + 1) * H],
                                          x1t[:])
                    x1q = wp.tile([128, H], F8E4, tag="x1q")
                    nc.scalar.activation(x1q[:], x1t[:], ACTF.Copy,
                                         scale=X1SCALE)
                    nc.sync.dma_start(cc_in[t * 128:(t + 1) * 128, :], x1q[:])
                    chunk_base += ncht

                    tper = NT // NAG
                    if (t + 1) % tper == 0:
                        g = (t + 1) // tper - 1
                        rows = NOWN // NAG
                        nc.gpsimd.collective_compute(
                            "AllGather", ALU.bypass,
                            replica_groups=[list(range(NCORES))],
                            ins=[cc_in[g * rows:(g + 1) * rows, :].opt()],
                            outs=[cc_out[g * NCORES * rows:
                                         (g + 1) * NCORES * rows, :].opt()])

            # ================= Layer 2 =================
            with (
                tc.tile_pool(name="w2", bufs=2) as wp2,
                tc.tile_pool(name="ps2", bufs=2, space="PSUM") as pp2,
            ):
                tb2 = np.concatenate([[0], np.cumsum(S2 // 128)]).astype(int)
                bt_all = wp2.tile([128, NT * H], BF, tag="btall", bufs=1)

                # phases A (slab 0) then B (slab 1): the slab-1 gathers wait
                # on the 2nd AllGather; keeping them out of the gpsimd stream
                # until all slab-0 gathers are issued hides that latency.
                for g in range(NAG):
                    for t in range(NT):
                        gsl = [x for x in sched2[t] if x[0] == g]
                        (_, gc0, gc1) = gsl[0]
                        nch_g = gc1 - gc0
                        base = int(tb2[t])
                        x1s_g = wp2.tile([128, K2m * H], F8E4, tag="x1s")
                        oh2_t = wp2.tile([128, K2m * 128], F8E4, tag="oh2t")
                        nc.sync.dma_start(
                            oh2_t[:, :nch_g * 128],
                            oh2_d[:, (base + gc0) * 128:
                                  (base + gc1) * 128])
                        sub_gather(x1s_g, cc_out[:], src2_sb,
                                   base + gc0, nch_g, H, gblk2)
                        bt0 = pp2.tile([128, 384], FP32, tag="btp", bufs=2)
                        bt1 = pp2.tile([128, 384], FP32, tag="btq", bufs=2)
                        for ci in range(nch_g):
                            x1s_c = x1s_g[:, ci * H:(ci + 1) * H]
                            oh_c = oh2_t[:, ci * 128:(ci + 1) * 128]
                            nc.tensor.matmul(
                                bt0[:], oh_c, x1s_c[:, 0:384],
                                start=(ci == 0), stop=(ci == nch_g - 1))
                            nc.tensor.matmul(
                                bt1[:], oh_c, x1s_c[:, 384:768],
                                start=(ci == 0), stop=(ci == nch_g - 1))
                        bt_t = bt_all[:, t * H:(t + 1) * H]
                        if g == 0:
                            nc.vector.tensor_copy(bt_t[:, 0:384], bt0[:])
                            nc.vector.tensor_copy(bt_t[:, 384:768], bt1[:])
                        else:
                            nc.vector.tensor_add(bt_t[:, 0:384],
                                                 bt_t[:, 0:384], bt0[:])
                            nc.vector.tensor_add(bt_t[:, 384:768],
                                                 bt_t[:, 384:768], bt1[:])

                for w in range(4):
                    btT_sb = wp2.tile([128, 6 * 512], BF, tag="btTsb")
                    x1wT_sb = wp2.tile([128, 6 * 512], BF, tag="x1wT")
                    for tt in range(4):
                        t = w * 4 + tt
                        x1o_t = x1own_sb[:, t * H:(t + 1) * H]
                        bt_t = bt_all[:, t * H:(t + 1) * H]
                        for k in range(6):
                            trp = pp2.tile([128, 128], BF, tag="trp",
                                           bufs=2)
                            nc.tensor.transpose(
                                trp[:], bt_t[:, k * 128:(k + 1) * 128],
                                ident_sb[:])
                            nc.scalar.copy(
                                btT_sb[:, k * 512 + tt * 128:
                                       k * 512 + (tt + 1) * 128], trp[:])
                            trq = pp2.tile([128, 128], BF, tag="trp",
                                           bufs=2)
                            nc.tensor.transpose(
                                trq[:], x1o_t[:, k * 128:(k + 1) * 128],
                                ident_sb[:])
                            nc.scalar.copy(
                                x1wT_sb[:, k * 512 + tt * 128:
                                        k * 512 + (tt + 1) * 128], trq[:])

                    for j in range(6):
                        aps = pp2.tile([128, 512], FP32, tag="agg2")
                        first = True
                        for k in range(6):
                            nc.tensor.matmul(
                                aps[:],
                                lint_sb[:, k * H + j * 128:
                                        k * H + (j + 1) * 128],
                                btT_sb[:, k * 512:(k + 1) * 512],
                                start=first, stop=False)
                            first = False
                            nc.tensor.matmul(
                                aps[:],
                                selft_sb[:, k * H + j * 128:
                                         k * H + (j + 1) * 128],
                                x1wT_sb[:, k * 512:(k + 1) * 512],
                                start=False, stop=False)
                        nc.tensor.matmul(
                            aps[:], b2_sb[:, j * 128:(j + 1) * 128],
                            degones_sb[:, w * 512:(w + 1) * 512],
                            start=False, stop=True)
                        x2 = wp2.tile([128, 512], FP32, tag="x2")
                        nc.scalar.activation(x2[:], aps[:], ACTF.Relu)
                        red = wp2.tile([128, 1], FP32, tag="red")
                        nc.vector.reduce_sum(red[:], x2[:], axis=AX.X)
                        nc.vector.tensor_add(pooled_sb[:, j:j + 1],
                                             pooled_sb[:, j:j + 1], red[:])

            nc.sync.dma_start(pooled_d[:], pooled_sb[:])

    nc.compile()
    return nc


def make_in_maps(inputs, sch):
    x = np.asarray(inputs["x"], np.float32)
    relwt = np.ascontiguousarray(
        np.asarray(inputs["rel_W"], np.float32).transpose(0, 2, 1)).astype(BF16)
    # lint is applied to the fp8-scaled aggregate: fold 1/X1SCALE here.
    lint = np.ascontiguousarray(
        np.asarray(inputs["mp_lin_W"], np.float32).T / X1SCALE).astype(BF16)
    selft = np.ascontiguousarray(
        np.asarray(inputs["mp_self_W"], np.float32).T).astype(BF16)
    b2 = np.stack([np.asarray(inputs["mp_lin_b"], np.float32),
                   np.asarray(inputs["mp_self_b"], np.float32)])
    xbm = x.astype(BF16)
    iota = np.tile(np.arange(128, dtype=np.float32), (128, 1))
    in_maps = []
    for c in range(NCORES):
        cd = sch["cores"][c]
        xown = np.ascontiguousarray(
            x[cd["own"]].reshape(NT, 128, DIN).transpose(1, 0, 2)
            .reshape(128, NT * DIN)).astype(F8)
        in_maps.append(dict(
            xb=xbm, xown=xown, relwt=relwt,
            relb=np.asarray(inputs["rel_b"], np.float32),
            lint=lint, selft=selft, b2=b2, degones=cd["degones"],
            src1=cd["src1"], slot1=cd["slot1"],
            rn1=cd["rn1"], ohuT1=cd["ohuT1"],
            src2=cd["src2"], oh2=cd["oh2"],
            iota=iota, ident=np.eye(128, dtype=BF16)))
    return in_maps


def prep_from_inputs(inputs):
    ei = np.asarray(inputs["edge_index"], np.int64)
    et = np.asarray(inputs["edge_type"], np.int64)
    nrm = np.asarray(inputs["norm_constants"], np.float32)
    invnc = (1.0 / nrm)[et].astype(np.float32)
    return _prep(ei[0], ei[1], et, invnc)


def kernel(**inputs) -> np.ndarray:
    out_W = np.asarray(inputs["out_W"], np.float32)
    out_b = np.asarray(inputs["out_b"], np.float32)

    import time as _t
    t0 = _t.time()
    sch = prep_from_inputs(inputs)
    print(f"[kernel] prep {_t.time()-t0:.1f}s  E1={sch['E1']} E2={sch['E2']}",
          flush=True)
    t0 = _t.time()
    nc = _build(sch)
    print(f"[kernel] build+compile {_t.time()-t0:.1f}s", flush=True)

    in_maps = make_in_maps(inputs, sch)

    t0 = _t.time()
    tmpdir = os.environ.get("TRACE_TMPDIR")
    if tmpdir:
        os.makedirs(tmpdir, exist_ok=True)
    res = bass_utils.run_bass_kernel_spmd(
        nc, in_maps, core_ids=list(range(NCORES)), tmpdir=tmpdir)
    print(f"[kernel] run {_t.time()-t0:.1f}s", flush=True)
    if res.instructions_and_trace is not None:
        print(f"[kernel] trace: {res.instructions_and_trace[1]}", flush=True)
    if res.profile_json is not None:
        print(f"[kernel] profile_json: {res.profile_json}", flush=True)

    pooled = np.zeros(H, np.float64)
    for c in range(NCORES):
        p = res.results[c]["pooled"]  # [128, 6]
        pooled += p.T.reshape(-1).astype(np.float64)  # h = j*128 + p

    kernel._last_exec_ns = res.exec_time_ns

    out = (pooled / N).astype(np.float32) @ out_W.T + out_b
    return out.astype(np.float32)


# revision 18
# speedup vs baseline: 1.2558x; 1.2558x over previous
"""Trainium2 Bass kernel for a 2-layer relational GNN (EvalNet).

Strategy (v2): shard by destination node with a *balanced* node->
(core,tile,slot) assignment (equalizes per-(core,tile,relation) edge
counts, minimizing gather padding). Layer-1 aggregations are core-local.

Layer 1 per dst tile: gather x[src] rows (bf16) per edge; reconstruct
x[dst] per edge ON-CHIP as a one-hot matmul against the tile's own x
(eliminating the dst gather entirely); edge weight via DVE
scalar_tensor_tensor with row-accumulate; weighted one-hot scatter into
per-relation PSUM; apply rel_W after aggregation.

x1 is AllGathered in fp8 (nag=2 slabs, overlapped with L1). Layer 2
gathers fp8 x1 rows per edge (src-deduped per (tile,slab) with
multi-hot one-hots streamed from host), scatters with oh2 as the
stationary matmul operand, transposes the aggregate on-chip, and
applies mp_lin/mp_self after aggregation. The own-node (self) path uses
the locally kept bf16 x1 (no gather). pooled is reduced on-chip;
the final 5-way projection of the 768-d pooled mean is on host.
"""

import os
import sys

sys.path.insert(0, "/opt/trn_rl_repo")

import numpy as np
import ml_dtypes

import concourse.bacc as bacc
import concourse.tile as tile
import concourse.mybir as mybir
from concourse import bass_utils
from concourse.library_config import mlp as mlp_lib

BF16 = ml_dtypes.bfloat16
F8 = ml_dtypes.float8_e4m3

N = 16384
E = 262144
R = 9
DIN = 384
H = 768
NCLS = 5
NCORES = 8
NOWN = N // NCORES          # 2048 nodes per core
NT = NOWN // 128            # 16 dst tiles of 128 per core
NBIN = NCORES * NT          # 128 (core,tile) bins
NAG = 2                     # AllGather slabs
X1SCALE = 0.125             # x1 -> fp8 scale (undone in lint host-side)

FP32 = mybir.dt.float32
BF = mybir.dt.bfloat16
F8E4 = mybir.dt.float8e4
I16 = mybir.dt.int16
AX = mybir.AxisListType
ALU = mybir.AluOpType
ACTF = mybir.ActivationFunctionType


def _wrap16(ids):
    """int16 index layout for dma_gather: [128, n/16], element i at
    [i%16 (+16r replicas), i//16]."""
    a = np.asarray(ids, np.int16).reshape(-1, 16).T  # [16, n/16]
    return np.ascontiguousarray(np.tile(a, (8, 1)))


def _roundup(x, m):
    return (x + m - 1) // m * m


def _balance_bins(dst, et):
    """Assign nodes to NBIN bins of 128 slots, balancing the 9-dim
    in-degree vectors. Returns pos[node] = global position (bin*128+slot)."""
    deg = np.zeros((N, R), np.int64)
    np.add.at(deg, (dst, et), 1)
    order = np.argsort(-deg.sum(1), kind="stable")
    bin_cnt = np.zeros((NBIN, R), np.float64)
    bin_n = np.zeros(NBIN, np.int64)
    pos = np.zeros(N, np.int64)
    for n in order:
        d = deg[n].astype(np.float64)
        # min over open bins of dot(current load, node's degree vector)
        cost = bin_cnt @ d + 1e-6 * bin_cnt.sum(1)
        cost[bin_n >= 128] = np.inf
        b = int(np.argmin(cost))
        pos[n] = b * 128 + bin_n[b]
        bin_n[b] += 1
        bin_cnt[b] += d
    assert (bin_n == 128).all()
    return pos


def _ag_row(pos):
    """x1 table row (after slab-major AllGather) for global position pos."""
    c, loc = pos // NOWN, pos % NOWN
    g, l = loc // (NOWN // NAG), loc % (NOWN // NAG)
    return g * (NCORES * (NOWN // NAG)) + c * (NOWN // NAG) + l


def _prep(src, dst, et, invnc):
    pos = _balance_bins(dst, et)
    dcore = pos[dst] // NOWN
    dtile = (pos[dst] % NOWN) // 128
    dslot = pos[dst] % 128
    # source AG slab (for L2 gather gating): slab of the src node's tile
    sslab = (pos[src] % NOWN) // (NOWN // NAG)

    per_core = [np.nonzero(dcore == c)[0] for c in range(NCORES)]

    # ---------- L1: group by (tile, rel) ----------
    counts1 = np.zeros((NCORES, NT, R), np.int64)
    for c in range(NCORES):
        e = per_core[c]
        np.add.at(counts1[c], (dtile[e], et[e]), 1)
    K1 = _roundup(counts1.max(axis=0), 128)     # [NT, R]
    S1 = K1.sum(axis=1)                          # [NT]
    E1 = int(S1.sum())
    NCH1 = E1 // 128

    sched1 = []
    for t in range(NT):
        gs, c0 = [], 0
        for r in range(R):
            nch = int(K1[t, r]) // 128
            if nch:
                gs.append((r, c0, c0 + nch))
                c0 += nch
        sched1.append(gs)

    # ---------- L2: group by (tile, slab), dedup by src ----------
    # distinct-src counts per (core,tile,slab)
    counts2 = np.zeros((NCORES, NT, NAG), np.int64)
    groups2 = {}
    for c in range(NCORES):
        e = per_core[c]
        for t in range(NT):
            sel_t = e[dtile[e] == t]
            for g in range(NAG):
                es = sel_t[sslab[sel_t] == g]
                srcs, inv_idx = np.unique(src[es], return_inverse=True)
                counts2[c, t, g] = srcs.size
                groups2[(c, t, g)] = (srcs, inv_idx, es)
    K2 = _roundup(np.maximum(counts2.max(axis=0), 1), 128)  # [NT, NAG]
    S2 = K2.sum(axis=1)
    E2 = int(S2.sum())
    NCH2 = E2 // 128

    sched2 = []   # per tile: [(g, chunk0, chunk1)]
    for t in range(NT):
        gs, c0 = [], 0
        for g in range(NAG):
            nch = int(K2[t, g]) // 128
            gs.append((g, c0, c0 + nch))
            c0 += nch
        sched2.append(gs)

    cores = []
    for c in range(NCORES):
        src1 = np.zeros(E1, np.int64)
        slot1 = np.full(E1, -1.0, np.float32)
        inv1 = np.zeros(E1, np.float32)
        rel1 = np.full(E1, -1, np.int64)
        off = 0
        e_all = per_core[c]
        for t in range(NT):
            for r in range(R):
                k = int(K1[t, r])
                if k == 0:
                    continue
                es = e_all[(dtile[e_all] == t) & (et[e_all] == r)]
                n = es.size
                src1[off:off + n] = src[es]
                slot1[off:off + n] = dslot[es]
                inv1[off:off + n] = invnc[es]
                rel1[off:off + n] = r
                off += k
        assert off == E1

        # ohuT1[s, e] one-hot of slot (fp8), rn1[e%128, chunk*R + r]
        ohuT1 = np.zeros((128, E1), F8)
        val = slot1 >= 0
        idx = np.nonzero(val)[0]
        ohuT1[slot1[idx].astype(np.int64), idx] = inv1[idx]
        rn1 = np.zeros((128, NCH1 * R), BF16)
        rn1[idx % 128, (idx // 128) * R + rel1[idx]] = 1.0

        src2 = np.zeros(E2, np.int64)
        oh2 = np.zeros((128, E2), np.float32)
        off = 0
        for t in range(NT):
            for g in range(NAG):
                k = int(K2[t, g])
                srcs, inv_idx, es = groups2[(c, t, g)]
                n = srcs.size
                src2[off:off + n] = _ag_row(pos[srcs])
                np.add.at(oh2, (dslot[es], off + inv_idx), 1.0)
                off += k
        assert off == E2
        oh2 = oh2.astype(F8)
        # reshape oh2 to [128 p=e%128, chunk*128 + s]
        oh2v = np.zeros((128, NCH2 * 128), F8)
        er = np.arange(E2)
        oh2v[:, :] = oh2.T.reshape(NCH2, 128, 128).transpose(1, 0, 2).reshape(128, -1)

        def wrap128(v):
            o = np.zeros((128, v.size // 128), np.float32)
            p = np.arange(v.size)
            o[p % 128, p // 128] = v
            return o

        own = np.argsort(pos)[c * NOWN:(c + 1) * NOWN]  # node ids in (t,s) order
        deg2 = np.zeros(NOWN, np.float32)
        lp = pos[dst[e_all]] - c * NOWN
        np.add.at(deg2, lp, 1.0)
        degones = np.stack([deg2, np.ones(NOWN, np.float32)])

        cores.append(dict(
            src1=_wrap16(src1), slot1=wrap128(slot1),
            rn1=rn1, ohuT1=ohuT1,
            src2=_wrap16(src2), oh2=oh2v,
            degones=degones, own=own,
        ))

    return dict(E1=E1, NCH1=NCH1, S1=S1, sched1=sched1,
                E2=E2, NCH2=NCH2, S2=S2, sched2=sched2,
                K2m=int(K2.max()) // 128, cores=cores, pos=pos)


def _build(sch):
    E1, NCH1, S1, sched1 = sch["E1"], sch["NCH1"], sch["S1"], sch["sched1"]
    E2, NCH2, S2, sched2 = sch["E2"], sch["NCH2"], sch["S2"], sch["sched2"]
    G1 = max(int(s) for s in S1) // 128
    G2 = max(int(s) for s in S2) // 128
    K2m = sch["K2m"]
    gblk = int(os.environ.get("GBLK", 8))
    gblk2 = int(os.environ.get("GBLK2", 8))
    scratch = int(os.environ.get("DMA_SCRATCH", 16384))

    nc = bacc.Bacc("TRN2", target_bir_lowering=False, debug=False,
                   num_devices=NCORES, dynamic_dma_scratch_size=scratch)

    xb_d = nc.dram_tensor("xb", [N, DIN], BF, kind="ExternalInput")
    xown_d = nc.dram_tensor("xown", [128, NT * DIN], F8E4, kind="ExternalInput")
    relwt_d = nc.dram_tensor("relwt", [R, DIN, H], BF, kind="ExternalInput")
    relb_d = nc.dram_tensor("relb", [R, H], FP32, kind="ExternalInput")
    lint_d = nc.dram_tensor("lint", [H, H], BF, kind="ExternalInput")
    selft_d = nc.dram_tensor("selft", [H, H], BF, kind="ExternalInput")
    b2_d = nc.dram_tensor("b2", [2, H], FP32, kind="ExternalInput")
    degones_d = nc.dram_tensor("degones", [2, NOWN], FP32, kind="ExternalInput")
    src1_d = nc.dram_tensor("src1", [128, E1 // 16], I16, kind="ExternalInput")
    slot1_d = nc.dram_tensor("slot1", [128, NCH1], FP32, kind="ExternalInput")
    rn1_d = nc.dram_tensor("rn1", [128, NCH1 * R], BF, kind="ExternalInput")
    ohuT1_d = nc.dram_tensor("ohuT1", [128, E1], F8E4, kind="ExternalInput")
    src2_d = nc.dram_tensor("src2", [128, E2 // 16], I16, kind="ExternalInput")
    oh2_d = nc.dram_tensor("oh2", [128, NCH2 * 128], F8E4, kind="ExternalInput")
    iota_d = nc.dram_tensor("iota", [128, 128], FP32, kind="ExternalInput")
    ident_d = nc.dram_tensor("ident", [128, 128], BF, kind="ExternalInput")
    pooled_d = nc.dram_tensor("pooled", [128, 6], FP32, kind="ExternalOutput")

    with tile.TileContext(nc) as tc:
        nc.gpsimd.load_library(mlp_lib)
        with (
            tc.tile_pool(name="const", bufs=1) as cp,
            tc.tile_pool(name="dram", bufs=1, space="DRAM") as dp,
        ):
            # ---- metadata / small constants (gate first gathers) ----
            src1_sb = cp.tile([128, E1 // 16], I16)
            nc.sync.dma_start(src1_sb[:], src1_d[:])
            slot1_sb = cp.tile([128, NCH1], FP32)
            nc.sync.dma_start(slot1_sb[:], slot1_d[:])
            rn1_sb = cp.tile([128, NCH1 * R], BF)
            nc.sync.dma_start(rn1_sb[:], rn1_d[:])
            iota_sb = cp.tile([128, 128], FP32)
            nc.sync.dma_start(iota_sb[:], iota_d[:])
            ident_sb = cp.tile([128, 128], BF)
            nc.sync.dma_start(ident_sb[:], ident_d[:])
            src2_sb = cp.tile([128, E2 // 16], I16)
            nc.sync.dma_start(src2_sb[:], src2_d[:])
            xown_sb = cp.tile([128, NT * DIN], F8E4)
            nc.sync.dma_start(xown_sb[:], xown_d[:])
            relb_sb = cp.tile([R, H], FP32)
            nc.sync.dma_start(relb_sb[:], relb_d[:])
            b2_sb = cp.tile([2, H], FP32)
            nc.sync.dma_start(b2_sb[:], b2_d[:])
            degones_sb = cp.tile([2, NOWN], FP32)
            nc.sync.dma_start(degones_sb[:], degones_d[:])
            lint_sb = cp.tile([128, 6 * H], BF)
            selft_sb = cp.tile([128, 6 * H], BF)
            nc.sync.dma_start(
                lint_sb.rearrange("p (k h) -> p k h", h=H)[:, :, :],
                lint_d.rearrange("(k p) h -> p k h", p=128)[:, :, :])
            nc.sync.dma_start(
                selft_sb.rearrange("p (k h) -> p k h", h=H)[:, :, :],
                selft_d.rearrange("(k p) h -> p k h", p=128)[:, :, :])
            pooled_sb = cp.tile([128, 6], FP32)
            nc.vector.memset(pooled_sb[:], 0.0)
            # x1 kept locally (bf16) + transposed-on-demand for L2 self path
            x1own_sb = cp.tile([128, NT * H], BF)

            cc_in = dp.tile([NOWN, H], F8E4)
            cc_out = dp.tile([N, H], F8E4)

            def sub_gather(dst_tile, src_ap, idx_sb, chunk0, nchunks, elem,
                           blk, out_chunk0=0):
                v3 = dst_tile.rearrange("p (c d) -> p c d", d=elem)
                for b0 in range(0, nchunks, blk):
                    b1 = min(b0 + blk, nchunks)
                    col = (chunk0 + b0) * 8
                    nc.gpsimd.dma_gather(
                        v3[:, out_chunk0 + b0:out_chunk0 + b1, :], src_ap,
                        idx_sb[:, col:col + (b1 - b0) * 8],
                        (b1 - b0) * 128, (b1 - b0) * 128, elem,
                        single_packet=False)

            # ================= Layer 1 =================
            with (
                tc.tile_pool(name="w1c", bufs=1) as wc,
                tc.tile_pool(name="w1", bufs=2) as wp,
                tc.tile_pool(name="ps1", bufs=2, space="PSUM") as pp,
            ):
                relwt_sb = wc.tile([128, R * 3 * H], BF)
                nc.sync.dma_start(
                    relwt_sb.rearrange("p (r k h) -> p r k h", k=3, h=H)[:, :, :, :],
                    relwt_d.rearrange("r (k p) h -> p r k h", p=128)[:, :, :, :])

                chunk_base = 0
                for t in range(NT):
                    ncht = int(S1[t]) // 128
                    xs_g = wp.tile([128, G1 * DIN], BF, tag="xs")
                    sub_gather(xs_g, xb_d[:], src1_sb, chunk_base, ncht, DIN,
                               gblk)
                    ohuT_t = wp.tile([128, G1 * 128], F8E4, tag="ohuT")
                    nc.sync.dma_start(
                        ohuT_t[:, :ncht * 128],
                        ohuT1_d[:, chunk_base * 128:(chunk_base + ncht) * 128])

                    art_sb = wp.tile([128, R * 3 * 128], BF, tag="artsb")
                    ct_ps = pp.tile([R, 128], FP32, tag="ct", bufs=1)

                    first_ct = True
                    for (r, gc0, gc1) in sched1[t]:
                        art_ps = pp.tile([128, 3 * 128], FP32, tag="art",
                                         bufs=2)
                        for ci in range(gc0, gc1):
                            gci = chunk_base + ci
                            xs_c = xs_g[:, ci * DIN:(ci + 1) * DIN]
                            # reconstruct x_dst rows for this chunk on PE
                            xd_ps = pp.tile([128, DIN], FP32, tag="xd",
                                            bufs=2)
                            nc.tensor.matmul(
                                xd_ps[:],
                                ohuT_t[:, ci * 128:(ci + 1) * 128],
                                xown_sb[:, t * DIN:(t + 1) * DIN],
                                start=True, stop=True)
                            norm = wp.tile([128, 1], FP32, tag="norm", bufs=4)
                            prod = wp.tile([128, DIN], BF, tag="prod", bufs=4)
                            nc.vector.scalar_tensor_tensor(
                                prod[:], xs_c, 1.0, xd_ps[:],
                                ALU.mult, ALU.mult, accum_out=norm[:])
                            ohw = wp.tile([128, 128], BF, tag="ohw", bufs=4)
                            nc.vector.tensor_scalar(
                                ohw[:], iota_sb[:], slot1_sb[:, gci:gci + 1],
                                norm[:], ALU.is_equal, ALU.mult)
                            nc.tensor.matmul(
                                ct_ps[:], rn1_sb[:, gci * R:(gci + 1) * R],
                                ohw[:], start=first_ct,
                                stop=(ci == sched1[t][-1][2] - 1))
                            first_ct = False
                            for k in range(3):
                                nc.tensor.matmul(
                                    art_ps[:, k * 128:(k + 1) * 128],
                                    xs_c[:, k * 128:(k + 1) * 128],
                                    ohw[:], start=(ci == gc0 and k == 0),
                                    stop=(ci == gc1 - 1 and k == 2))
                        nc.scalar.copy(
                            art_sb[:, r * 384:(r + 1) * 384], art_ps[:])

                    ct_sb = wp.tile([R, 128], FP32, tag="ctsb")
                    nc.vector.tensor_copy(ct_sb[:], ct_ps[:])

                    x1t = wp.tile([128, H], BF, tag="x1t")
                    for s in range(2):
                        mps = pp.tile([128, 384], FP32, tag="mps", bufs=2)
                        first = True
                        for (r, _, _) in sched1[t]:
                            for k in range(3):
                                nc.tensor.matmul(
                                    mps[:],
                                    art_sb[:, r * 384 + k * 128:
                                           r * 384 + (k + 1) * 128],
                                    relwt_sb[:, (r * 3 + k) * H + s * 384:
                                             (r * 3 + k) * H + (s + 1) * 384],
                                    start=first, stop=False)
                                first = False
                        nc.tensor.matmul(mps[:], ct_sb[:],
                                         relb_sb[:, s * 384:(s + 1) * 384],
                                         start=False, stop=True)
                        nc.scalar.activation(x1t[:, s * 384:(s + 1) * 384],
                                             mps[:], ACTF.Relu)
                    nc.vector.tensor_copy(x1own_sb[:, t * H:(t + 1) * H],
                                          x1t[:])
                    x1q = wp.tile([128, H], F8E4, tag="x1q")
                    nc.scalar.activation(x1q[:], x1t[:], ACTF.Copy,
                                         scale=X1SCALE)
                    nc.sync.dma_start(cc_in[t * 128:(t + 1) * 128, :], x1q[:])
                    chunk_base += ncht

                    tper = NT // NAG
                    if (t + 1) % tper == 0:
                        g = (t + 1) // tper - 1
                        rows = NOWN // NAG
                        nc.gpsimd.collective_compute(
                            "AllGather", ALU.bypass,
                            replica_groups=[list(range(NCORES))],
                            ins=[cc_in[g * rows:(g + 1) * rows, :].opt()],
                            outs=[cc_out[g * NCORES * rows:
                                         (g + 1) * NCORES * rows, :].opt()])

            # ================= Layer 2 =================
            with (
                tc.tile_pool(name="w2", bufs=2) as wp2,
                tc.tile_pool(name="ps2", bufs=2, space="PSUM") as pp2,
            ):
                tb2 = np.concatenate([[0], np.cumsum(S2 // 128)]).astype(int)
                bt_all = wp2.tile([128, NT * H], BF, tag="btall", bufs=1)

                # phases A (slab 0) then B (slab 1): the slab-1 gathers wait
                # on the 2nd AllGather; keeping them out of the gpsimd stream
                # until all slab-0 gathers are issued hides that latency.
                def l2_tile(g, t):
                        gsl = [x for x in sched2[t] if x[0] == g]
                        (_, gc0, gc1) = gsl[0]
                        nch_g = gc1 - gc0
                        base = int(tb2[t])
                        x1s_g = wp2.tile([128, K2m * H], F8E4, tag="x1s")
                        oh2_t = wp2.tile([128, K2m * 128], F8E4, tag="oh2t")
                        nc.sync.dma_start(
                            oh2_t[:, :nch_g * 128],
                            oh2_d[:, (base + gc0) * 128:
                                  (base + gc1) * 128])
                        sub_gather(x1s_g, cc_out[:], src2_sb,
                                   base + gc0, nch_g, H, gblk2)
                        bt0 = pp2.tile([128, 384], FP32, tag="btp", bufs=2)
                        bt1 = pp2.tile([128, 384], FP32, tag="btq", bufs=2)
                        for ci in range(nch_g):
                            x1s_c = x1s_g[:, ci * H:(ci + 1) * H]
                            oh_c = oh2_t[:, ci * 128:(ci + 1) * 128]
                            nc.tensor.matmul(
                                bt0[:], oh_c, x1s_c[:, 0:384],
                                start=(ci == 0), stop=(ci == nch_g - 1))
                            nc.tensor.matmul(
                                bt1[:], oh_c, x1s_c[:, 384:768],
                                start=(ci == 0), stop=(ci == nch_g - 1))
                        bt_t = bt_all[:, t * H:(t + 1) * H]
                        if g == 0:
                            nc.vector.tensor_copy(bt_t[:, 0:384], bt0[:])
                            nc.vector.tensor_copy(bt_t[:, 384:768], bt1[:])
                        else:
                            nc.vector.tensor_add(bt_t[:, 0:384],
                                                 bt_t[:, 0:384], bt0[:])
                            nc.vector.tensor_add(bt_t[:, 384:768],
                                                 bt_t[:, 384:768], bt1[:])

                for t in range(NT):
                    l2_tile(0, t)
                for w in range(4):
                    for tt in range(4):
                        l2_tile(1, w * 4 + tt)
                    btT_sb = wp2.tile([128, 6 * 512], BF, tag="btTsb")
                    x1wT_sb = wp2.tile([128, 6 * 512], BF, tag="x1wT")
                    for tt in range(4):
                        t = w * 4 + tt
                        x1o_t = x1own_sb[:, t * H:(t + 1) * H]
                        bt_t = bt_all[:, t * H:(t + 1) * H]
                        for k in range(6):
                            trp = pp2.tile([128, 128], BF, tag="trp",
                                           bufs=2)
                            nc.tensor.transpose(
                                trp[:], bt_t[:, k * 128:(k + 1) * 128],
                                ident_sb[:])
                            nc.scalar.copy(
                                btT_sb[:, k * 512 + tt * 128:
                                       k * 512 + (tt + 1) * 128], trp[:])
                            trq = pp2.tile([128, 128], BF, tag="trp",
                                           bufs=2)
                            nc.tensor.transpose(
                                trq[:], x1o_t[:, k * 128:(k + 1) * 128],
                                ident_sb[:])
                            nc.scalar.copy(
                                x1wT_sb[:, k * 512 + tt * 128:
                                        k * 512 + (tt + 1) * 128], trq[:])

                    for j in range(6):
                        aps = pp2.tile([128, 512], FP32, tag="agg2")
                        first = True
                        for k in range(6):
                            nc.tensor.matmul(
                                aps[:],
                                lint_sb[:, k * H + j * 128:
                                        k * H + (j + 1) * 128],
                                btT_sb[:, k * 512:(k + 1) * 512],
                                start=first, stop=False)
                            first = False
                            nc.tensor.matmul(
                                aps[:],
                                selft_sb[:, k * H + j * 128:
                                         k * H + (j + 1) * 128],
                                x1wT_sb[:, k * 512:(k + 1) * 512],
                                start=False, stop=False)
                        nc.tensor.matmul(
                            aps[:], b2_sb[:, j * 128:(j + 1) * 128],
                            degones_sb[:, w * 512:(w + 1) * 512],
                            start=False, stop=True)
                        x2 = wp2.tile([128, 512], FP32, tag="x2")
                        nc.scalar.activation(x2[:], aps[:], ACTF.Relu)
                        red = wp2.tile([128, 1], FP32, tag="red")
                        nc.vector.reduce_sum(red[:], x2[:], axis=AX.X)
                        nc.vector.tensor_add(pooled_sb[:, j:j + 1],
                                             pooled_sb[:, j:j + 1], red[:])

            nc.sync.dma_start(pooled_d[:], pooled_sb[:])

    nc.compile()
    return nc


def make_in_maps(inputs, sch):
    x = np.asarray(inputs["x"], np.float32)
    relwt = np.ascontiguousarray(
        np.asarray(inputs["rel_W"], np.float32).transpose(0, 2, 1)).astype(BF16)
    # lint is applied to the fp8-scaled aggregate: fold 1/X1SCALE here.
    lint = np.ascontiguousarray(
        np.asarray(inputs["mp_lin_W"], np.float32).T / X1SCALE).astype(BF16)
    selft = np.ascontiguousarray(
        np.asarray(inputs["mp_self_W"], np.float32).T).astype(BF16)
    b2 = np.stack([np.asarray(inputs["mp_lin_b"], np.float32),
                   np.asarray(inputs["mp_self_b"], np.float32)])
    xbm = x.astype(BF16)
    iota = np.tile(np.arange(128, dtype=np.float32), (128, 1))
    in_maps = []
    for c in range(NCORES):
        cd = sch["cores"][c]
        xown = np.ascontiguousarray(
            x[cd["own"]].reshape(NT, 128, DIN).transpose(1, 0, 2)
            .reshape(128, NT * DIN)).astype(F8)
        in_maps.append(dict(
            xb=xbm, xown=xown, relwt=relwt,
            relb=np.asarray(inputs["rel_b"], np.float32),
            lint=lint, selft=selft, b2=b2, degones=cd["degones"],
            src1=cd["src1"], slot1=cd["slot1"],
            rn1=cd["rn1"], ohuT1=cd["ohuT1"],
            src2=cd["src2"], oh2=cd["oh2"],
            iota=iota, ident=np.eye(128, dtype=BF16)))
    return in_maps


def prep_from_inputs(inputs):
    ei = np.asarray(inputs["edge_index"], np.int64)
    et = np.asarray(inputs["edge_type"], np.int64)
    nrm = np.asarray(inputs["norm_constants"], np.float32)
    invnc = (1.0 / nrm)[et].astype(np.float32)
    return _prep(ei[0], ei[1], et, invnc)


def kernel(**inputs) -> np.ndarray:
    out_W = np.asarray(inputs["out_W"], np.float32)
    out_b = np.asarray(inputs["out_b"], np.float32)

    import time as _t
    t0 = _t.time()
    sch = prep_from_inputs(inputs)
    print(f"[kernel] prep {_t.time()-t0:.1f}s  E1={sch['E1']} E2={sch['E2']}",
          flush=True)
    t0 = _t.time()
    nc = _build(sch)
    print(f"[kernel] build+compile {_t.time()-t0:.1f}s", flush=True)

    in_maps = make_in_maps(inputs, sch)

    t0 = _t.time()
    tmpdir = os.environ.get("TRACE_TMPDIR")
    if tmpdir:
        os.makedirs(tmpdir, exist_ok=True)
    res = bass_utils.run_bass_kernel_spmd(
        nc, in_maps, core_ids=list(range(NCORES)), tmpdir=tmpdir)
    print(f"[kernel] run {_t.time()-t0:.1f}s", flush=True)
    if res.instructions_and_trace is not None:
        print(f"[kernel] trace: {res.instructions_and_trace[1]}", flush=True)
    if res.profile_json is not None:
        print(f"[kernel] profile_json: {res.profile_json}", flush=True)

    pooled = np.zeros(H, np.float64)
    for c in range(NCORES):
        p = res.results[c]["pooled"]  # [128, 6]
        pooled += p.T.reshape(-1).astype(np.float64)  # h = j*128 + p

    kernel._last_exec_ns = res.exec_time_ns

    out = (pooled / N).astype(np.float32) @ out_W.T + out_b
    return out.astype(np.float32)


# revision 19
# speedup vs baseline: 1.2815x; 1.0205x over previous
"""Trainium2 Bass kernel for a 2-layer relational GNN (EvalNet).

Strategy (v2): shard by destination node with a *balanced* node->
(core,tile,slot) assignment (equalizes per-(core,tile,relation) edge
counts, minimizing gather padding). Layer-1 aggregations are core-local.

Layer 1 per dst tile: gather x[src] rows (bf16) per edge; reconstruct
x[dst] per edge ON-CHIP as a one-hot matmul against the tile's own x
(eliminating the dst gather entirely); edge weight via DVE
scalar_tensor_tensor with row-accumulate; weighted one-hot scatter into
per-relation PSUM; apply rel_W after aggregation.

x1 is AllGathered in fp8 (nag=2 slabs, overlapped with L1). Layer 2
gathers fp8 x1 rows per edge (src-deduped per (tile,slab) with
multi-hot one-hots streamed from host), scatters with oh2 as the
stationary matmul operand, transposes the aggregate on-chip, and
applies mp_lin/mp_self after aggregation. The own-node (self) path uses
the locally kept bf16 x1 (no gather). pooled is reduced on-chip;
the final 5-way projection of the 768-d pooled mean is on host.
"""

import os
import sys

sys.path.insert(0, "/opt/trn_rl_repo")

import numpy as np
import ml_dtypes

import concourse.bacc as bacc
import concourse.tile as tile
import concourse.mybir as mybir
from concourse import bass_utils
from concourse.library_config import mlp as mlp_lib

BF16 = ml_dtypes.bfloat16
F8 = ml_dtypes.float8_e4m3

N = 16384
E = 262144
R = 9
DIN = 384
H = 768
NCLS = 5
NCORES = 8
NOWN = N // NCORES          # 2048 nodes per core
NT = NOWN // 128            # 16 dst tiles of 128 per core
NBIN = NCORES * NT          # 128 (core,tile) bins
NAG = 2                     # AllGather slabs
X1SCALE = 0.125             # x1 -> fp8 scale (undone in lint host-side)

FP32 = mybir.dt.float32
BF = mybir.dt.bfloat16
F8E4 = mybir.dt.float8e4
I16 = mybir.dt.int16
AX = mybir.AxisListType
ALU = mybir.AluOpType
ACTF = mybir.ActivationFunctionType


def _wrap16(ids):
    """int16 index layout for dma_gather: [128, n/16], element i at
    [i%16 (+16r replicas), i//16]."""
    a = np.asarray(ids, np.int16).reshape(-1, 16).T  # [16, n/16]
    return np.ascontiguousarray(np.tile(a, (8, 1)))


def _roundup(x, m):
    return (x + m - 1) // m * m


def _balance_bins(dst, et):
    """Assign nodes to NBIN bins of 128 slots, balancing the 9-dim
    in-degree vectors. Returns pos[node] = global position (bin*128+slot)."""
    deg = np.zeros((N, R), np.int64)
    np.add.at(deg, (dst, et), 1)
    order = np.argsort(-deg.sum(1), kind="stable")
    bin_cnt = np.zeros((NBIN, R), np.float64)
    bin_n = np.zeros(NBIN, np.int64)
    pos = np.zeros(N, np.int64)
    for n in order:
        d = deg[n].astype(np.float64)
        # min over open bins of dot(current load, node's degree vector)
        cost = bin_cnt @ d + 1e-6 * bin_cnt.sum(1)
        cost[bin_n >= 128] = np.inf
        b = int(np.argmin(cost))
        pos[n] = b * 128 + bin_n[b]
        bin_n[b] += 1
        bin_cnt[b] += d
    assert (bin_n == 128).all()
    return pos


def _ag_row(pos):
    """x1 table row (after slab-major AllGather) for global position pos."""
    c, loc = pos // NOWN, pos % NOWN
    g, l = loc // (NOWN // NAG), loc % (NOWN // NAG)
    return g * (NCORES * (NOWN // NAG)) + c * (NOWN // NAG) + l


def _prep(src, dst, et, invnc):
    pos = _balance_bins(dst, et)
    dcore = pos[dst] // NOWN
    dtile = (pos[dst] % NOWN) // 128
    dslot = pos[dst] % 128
    # source AG slab (for L2 gather gating): slab of the src node's tile
    sslab = (pos[src] % NOWN) // (NOWN // NAG)

    per_core = [np.nonzero(dcore == c)[0] for c in range(NCORES)]

    # ---------- L1: group by (tile, rel) ----------
    counts1 = np.zeros((NCORES, NT, R), np.int64)
    for c in range(NCORES):
        e = per_core[c]
        np.add.at(counts1[c], (dtile[e], et[e]), 1)
    K1 = _roundup(counts1.max(axis=0), 128)     # [NT, R]
    S1 = K1.sum(axis=1)                          # [NT]
    E1 = int(S1.sum())
    NCH1 = E1 // 128

    sched1 = []
    for t in range(NT):
        gs, c0 = [], 0
        for r in range(R):
            nch = int(K1[t, r]) // 128
            if nch:
                gs.append((r, c0, c0 + nch))
                c0 += nch
        sched1.append(gs)

    # ---------- L2: group by (tile, slab), dedup by src ----------
    # distinct-src counts per (core,tile,slab)
    counts2 = np.zeros((NCORES, NT, NAG), np.int64)
    groups2 = {}
    for c in range(NCORES):
        e = per_core[c]
        for t in range(NT):
            sel_t = e[dtile[e] == t]
            for g in range(NAG):
                es = sel_t[sslab[sel_t] == g]
                srcs, inv_idx = np.unique(src[es], return_inverse=True)
                counts2[c, t, g] = srcs.size
                groups2[(c, t, g)] = (srcs, inv_idx, es)
    K2 = _roundup(np.maximum(counts2.max(axis=0), 1), 128)  # [NT, NAG]
    S2 = K2.sum(axis=1)
    E2 = int(S2.sum())
    NCH2 = E2 // 128

    sched2 = []   # per tile: [(g, chunk0, chunk1)]
    for t in range(NT):
        gs, c0 = [], 0
        for g in range(NAG):
            nch = int(K2[t, g]) // 128
            gs.append((g, c0, c0 + nch))
            c0 += nch
        sched2.append(gs)

    cores = []
    for c in range(NCORES):
        src1 = np.zeros(E1, np.int64)
        slot1 = np.full(E1, -1.0, np.float32)
        inv1 = np.zeros(E1, np.float32)
        rel1 = np.full(E1, -1, np.int64)
        off = 0
        e_all = per_core[c]
        for t in range(NT):
            for r in range(R):
                k = int(K1[t, r])
                if k == 0:
                    continue
                es = e_all[(dtile[e_all] == t) & (et[e_all] == r)]
                n = es.size
                src1[off:off + n] = src[es]
                slot1[off:off + n] = dslot[es]
                inv1[off:off + n] = invnc[es]
                rel1[off:off + n] = r
                off += k
        assert off == E1

        # ohuT1[s, e] one-hot of slot (fp8), rn1[e%128, chunk*R + r]
        ohuT1 = np.zeros((128, E1), F8)
        val = slot1 >= 0
        idx = np.nonzero(val)[0]
        ohuT1[slot1[idx].astype(np.int64), idx] = inv1[idx]
        rn1 = np.zeros((128, NCH1 * R), BF16)
        rn1[idx % 128, (idx // 128) * R + rel1[idx]] = 1.0

        src2 = np.zeros(E2, np.int64)
        oh2 = np.zeros((128, E2), np.float32)
        off = 0
        for t in range(NT):
            for g in range(NAG):
                k = int(K2[t, g])
                srcs, inv_idx, es = groups2[(c, t, g)]
                n = srcs.size
                src2[off:off + n] = _ag_row(pos[srcs])
                np.add.at(oh2, (dslot[es], off + inv_idx), 1.0)
                off += k
        assert off == E2
        oh2 = oh2.astype(F8)
        # reshape oh2 to [128 p=e%128, chunk*128 + s]
        oh2v = np.zeros((128, NCH2 * 128), F8)
        er = np.arange(E2)
        oh2v[:, :] = oh2.T.reshape(NCH2, 128, 128).transpose(1, 0, 2).reshape(128, -1)

        def wrap128(v):
            o = np.zeros((128, v.size // 128), np.float32)
            p = np.arange(v.size)
            o[p % 128, p // 128] = v
            return o

        own = np.argsort(pos)[c * NOWN:(c + 1) * NOWN]  # node ids in (t,s) order
        deg2 = np.zeros(NOWN, np.float32)
        lp = pos[dst[e_all]] - c * NOWN
        np.add.at(deg2, lp, 1.0)
        degones = np.stack([deg2, np.ones(NOWN, np.float32)])

        cores.append(dict(
            src1=_wrap16(src1), slot1=wrap128(slot1),
            rn1=rn1, ohuT1=ohuT1,
            src2=_wrap16(src2), oh2=oh2v,
            degones=degones, own=own,
        ))

    return dict(E1=E1, NCH1=NCH1, S1=S1, sched1=sched1,
                E2=E2, NCH2=NCH2, S2=S2, sched2=sched2,
                K2m=int(K2.max()) // 128, cores=cores, pos=pos)


def _build(sch):
    E1, NCH1, S1, sched1 = sch["E1"], sch["NCH1"], sch["S1"], sch["sched1"]
    E2, NCH2, S2, sched2 = sch["E2"], sch["NCH2"], sch["S2"], sch["sched2"]
    G1 = max(int(s) for s in S1) // 128
    G2 = max(int(s) for s in S2) // 128
    K2m = sch["K2m"]
    gblk = int(os.environ.get("GBLK", 8))
    gblk2 = int(os.environ.get("GBLK2", 8))
    scratch = int(os.environ.get("DMA_SCRATCH", 16384))

    nc = bacc.Bacc("TRN2", target_bir_lowering=False, debug=False,
                   num_devices=NCORES, dynamic_dma_scratch_size=scratch)

    xb_d = nc.dram_tensor("xb", [N, DIN], BF, kind="ExternalInput")
    xown_d = nc.dram_tensor("xown", [128, NT * DIN], F8E4, kind="ExternalInput")
    relwt_d = nc.dram_tensor("relwt", [R, DIN, H], F8E4, kind="ExternalInput")
    relb_d = nc.dram_tensor("relb", [R, H], FP32, kind="ExternalInput")
    lint_d = nc.dram_tensor("lint", [H, H], BF, kind="ExternalInput")
    selft_d = nc.dram_tensor("selft", [H, H], BF, kind="ExternalInput")
    b2_d = nc.dram_tensor("b2", [2, H], FP32, kind="ExternalInput")
    degones_d = nc.dram_tensor("degones", [2, NOWN], FP32, kind="ExternalInput")
    src1_d = nc.dram_tensor("src1", [128, E1 // 16], I16, kind="ExternalInput")
    slot1_d = nc.dram_tensor("slot1", [128, NCH1], FP32, kind="ExternalInput")
    rn1_d = nc.dram_tensor("rn1", [128, NCH1 * R], BF, kind="ExternalInput")
    ohuT1_d = nc.dram_tensor("ohuT1", [128, E1], F8E4, kind="ExternalInput")
    src2_d = nc.dram_tensor("src2", [128, E2 // 16], I16, kind="ExternalInput")
    oh2_d = nc.dram_tensor("oh2", [128, NCH2 * 128], F8E4, kind="ExternalInput")
    iota_d = nc.dram_tensor("iota", [128, 128], FP32, kind="ExternalInput")
    ident_d = nc.dram_tensor("ident", [128, 128], BF, kind="ExternalInput")
    pooled_d = nc.dram_tensor("pooled", [128, 6], FP32, kind="ExternalOutput")

    with tile.TileContext(nc) as tc:
        nc.gpsimd.load_library(mlp_lib)
        with (
            tc.tile_pool(name="const", bufs=1) as cp,
            tc.tile_pool(name="dram", bufs=1, space="DRAM") as dp,
        ):
            # ---- metadata / small constants (gate first gathers) ----
            src1_sb = cp.tile([128, E1 // 16], I16)
            nc.sync.dma_start(src1_sb[:], src1_d[:])
            slot1_sb = cp.tile([128, NCH1], FP32)
            nc.sync.dma_start(slot1_sb[:], slot1_d[:])
            rn1_sb = cp.tile([128, NCH1 * R], BF)
            nc.sync.dma_start(rn1_sb[:], rn1_d[:])
            iota_sb = cp.tile([128, 128], FP32)
            nc.sync.dma_start(iota_sb[:], iota_d[:])
            ident_sb = cp.tile([128, 128], BF)
            nc.sync.dma_start(ident_sb[:], ident_d[:])
            src2_sb = cp.tile([128, E2 // 16], I16)
            nc.sync.dma_start(src2_sb[:], src2_d[:])
            xown_sb = cp.tile([128, NT * DIN], F8E4)
            nc.sync.dma_start(xown_sb[:], xown_d[:])
            relb_sb = cp.tile([R, H], FP32)
            nc.sync.dma_start(relb_sb[:], relb_d[:])
            b2_sb = cp.tile([2, H], FP32)
            nc.sync.dma_start(b2_sb[:], b2_d[:])
            degones_sb = cp.tile([2, NOWN], FP32)
            nc.sync.dma_start(degones_sb[:], degones_d[:])
            lint_sb = cp.tile([128, 6 * H], BF)
            selft_sb = cp.tile([128, 6 * H], BF)
            nc.sync.dma_start(
                lint_sb.rearrange("p (k h) -> p k h", h=H)[:, :, :],
                lint_d.rearrange("(k p) h -> p k h", p=128)[:, :, :])
            nc.sync.dma_start(
                selft_sb.rearrange("p (k h) -> p k h", h=H)[:, :, :],
                selft_d.rearrange("(k p) h -> p k h", p=128)[:, :, :])
            pooled_sb = cp.tile([128, 6], FP32)
            nc.vector.memset(pooled_sb[:], 0.0)
            # x1 kept locally (bf16) + transposed-on-demand for L2 self path
            x1own_sb = cp.tile([128, NT * H], BF)

            cc_in = dp.tile([NOWN, H], F8E4)
            cc_out = dp.tile([N, H], F8E4)

            def sub_gather(dst_tile, src_ap, idx_sb, chunk0, nchunks, elem,
                           blk, out_chunk0=0):
                v3 = dst_tile.rearrange("p (c d) -> p c d", d=elem)
                for b0 in range(0, nchunks, blk):
                    b1 = min(b0 + blk, nchunks)
                    col = (chunk0 + b0) * 8
                    nc.gpsimd.dma_gather(
                        v3[:, out_chunk0 + b0:out_chunk0 + b1, :], src_ap,
                        idx_sb[:, col:col + (b1 - b0) * 8],
                        (b1 - b0) * 128, (b1 - b0) * 128, elem,
                        single_packet=False)

            # ================= Layer 1 =================
            with (
                tc.tile_pool(name="w1c", bufs=1) as wc,
                tc.tile_pool(name="w1", bufs=2) as wp,
                tc.tile_pool(name="ps1", bufs=2, space="PSUM") as pp,
            ):
                relwt_sb = wc.tile([128, R * 3 * H], F8E4)
                nc.sync.dma_start(
                    relwt_sb.rearrange("p (r k h) -> p r k h", k=3, h=H)[:, :, :, :],
                    relwt_d.rearrange("r (k p) h -> p r k h", p=128)[:, :, :, :])

                chunk_base = 0
                for t in range(NT):
                    ncht = int(S1[t]) // 128
                    xs_g = wp.tile([128, G1 * DIN], BF, tag="xs", bufs=3)
                    sub_gather(xs_g, xb_d[:], src1_sb, chunk_base, ncht, DIN,
                               gblk)
                    ohuT_t = wp.tile([128, G1 * 128], F8E4, tag="ohuT")
                    nc.sync.dma_start(
                        ohuT_t[:, :ncht * 128],
                        ohuT1_d[:, chunk_base * 128:(chunk_base + ncht) * 128])

                    art_sb = wp.tile([128, R * 3 * 128], F8E4, tag="artsb")
                    ct_ps = pp.tile([R, 128], FP32, tag="ct", bufs=1)

                    first_ct = True
                    for (r, gc0, gc1) in sched1[t]:
                        art_ps = pp.tile([128, 3 * 128], FP32, tag="art",
                                         bufs=2)
                        for ci in range(gc0, gc1):
                            gci = chunk_base + ci
                            xs_c = xs_g[:, ci * DIN:(ci + 1) * DIN]
                            # reconstruct x_dst rows for this chunk on PE
                            xd_ps = pp.tile([128, DIN], FP32, tag="xd",
                                            bufs=2)
                            nc.tensor.matmul(
                                xd_ps[:],
                                ohuT_t[:, ci * 128:(ci + 1) * 128],
                                xown_sb[:, t * DIN:(t + 1) * DIN],
                                start=True, stop=True)
                            norm = wp.tile([128, 1], FP32, tag="norm", bufs=4)
                            prod = wp.tile([128, DIN], BF, tag="prod", bufs=4)
                            nc.vector.scalar_tensor_tensor(
                                prod[:], xs_c, 1.0, xd_ps[:],
                                ALU.mult, ALU.mult, accum_out=norm[:])
                            ohw = wp.tile([128, 128], BF, tag="ohw", bufs=4)
                            nc.vector.tensor_scalar(
                                ohw[:], iota_sb[:], slot1_sb[:, gci:gci + 1],
                                norm[:], ALU.is_equal, ALU.mult)
                            nc.tensor.matmul(
                                ct_ps[:], rn1_sb[:, gci * R:(gci + 1) * R],
                                ohw[:], start=first_ct,
                                stop=(ci == sched1[t][-1][2] - 1))
                            first_ct = False
                            for k in range(3):
                                nc.tensor.matmul(
                                    art_ps[:, k * 128:(k + 1) * 128],
                                    xs_c[:, k * 128:(k + 1) * 128],
                                    ohw[:], start=(ci == gc0 and k == 0),
                                    stop=(ci == gc1 - 1 and k == 2))
                        nc.scalar.activation(
                            art_sb[:, r * 384:(r + 1) * 384], art_ps[:],
                            ACTF.Copy, scale=0.125)

                    ct_sb = wp.tile([R, 128], FP32, tag="ctsb")
                    nc.vector.tensor_copy(ct_sb[:], ct_ps[:])

                    x1t = wp.tile([128, H], BF, tag="x1t")
                    relwt_v = relwt_sb.rearrange("p (r k h) -> p r k h",
                                                 k=3, h=H)
                    for s in range(2):
                        mps = pp.tile([128, 384], FP32, tag="mps", bufs=2)
                        first = True
                        for (r, _, _) in sched1[t]:
                            nc.tensor.matmul(
                                mps[:],
                                art_sb[:, r * 384:r * 384 + 256].rearrange(
                                    "p (k m) -> p k m", k=2),
                                relwt_v[:, r, 0:2, s * 384:(s + 1) * 384],
                                start=first, stop=False,
                                perf_mode=mybir.MatmulPerfMode.DoubleRow)
                            first = False
                            nc.tensor.matmul(
                                mps[:],
                                art_sb[:, r * 384 + 256:(r + 1) * 384],
                                relwt_v[:, r, 2, s * 384:(s + 1) * 384],
                                start=False, stop=False)
                        nc.tensor.matmul(mps[:], ct_sb[:],
                                         relb_sb[:, s * 384:(s + 1) * 384],
                                         start=False, stop=True)
                        nc.scalar.activation(x1t[:, s * 384:(s + 1) * 384],
                                             mps[:], ACTF.Relu, scale=0.25)
                    nc.vector.tensor_copy(x1own_sb[:, t * H:(t + 1) * H],
                                          x1t[:])
                    x1q = wp.tile([128, H], F8E4, tag="x1q")
                    nc.scalar.activation(x1q[:], x1t[:], ACTF.Copy,
                                         scale=X1SCALE)
                    nc.sync.dma_start(cc_in[t * 128:(t + 1) * 128, :], x1q[:])
                    chunk_base += ncht

                    tper = NT // NAG
                    if (t + 1) % tper == 0:
                        g = (t + 1) // tper - 1
                        rows = NOWN // NAG
                        nc.gpsimd.collective_compute(
                            "AllGather", ALU.bypass,
                            replica_groups=[list(range(NCORES))],
                            ins=[cc_in[g * rows:(g + 1) * rows, :].opt()],
                            outs=[cc_out[g * NCORES * rows:
                                         (g + 1) * NCORES * rows, :].opt()])

            # ================= Layer 2 =================
            with (
                tc.tile_pool(name="w2", bufs=2) as wp2,
                tc.tile_pool(name="ps2", bufs=2, space="PSUM") as pp2,
            ):
                tb2 = np.concatenate([[0], np.cumsum(S2 // 128)]).astype(int)
                bt_all = wp2.tile([128, NT * H], BF, tag="btall", bufs=1)

                # phases A (slab 0) then B (slab 1): the slab-1 gathers wait
                # on the 2nd AllGather; keeping them out of the gpsimd stream
                # until all slab-0 gathers are issued hides that latency.
                def l2_tile(g, t):
                        gsl = [x for x in sched2[t] if x[0] == g]
                        (_, gc0, gc1) = gsl[0]
                        nch_g = gc1 - gc0
                        base = int(tb2[t])
                        x1s_g = wp2.tile([128, K2m * H], F8E4, tag="x1s")
                        oh2_t = wp2.tile([128, K2m * 128], F8E4, tag="oh2t")
                        nc.sync.dma_start(
                            oh2_t[:, :nch_g * 128],
                            oh2_d[:, (base + gc0) * 128:
                                  (base + gc1) * 128])
                        sub_gather(x1s_g, cc_out[:], src2_sb,
                                   base + gc0, nch_g, H, gblk2)
                        bt0 = pp2.tile([128, 384], FP32, tag="btp", bufs=2)
                        bt1 = pp2.tile([128, 384], FP32, tag="btq", bufs=2)
                        for ci in range(nch_g):
                            x1s_c = x1s_g[:, ci * H:(ci + 1) * H]
                            oh_c = oh2_t[:, ci * 128:(ci + 1) * 128]
                            nc.tensor.matmul(
                                bt0[:], oh_c, x1s_c[:, 0:384],
                                start=(ci == 0), stop=(ci == nch_g - 1))
                            nc.tensor.matmul(
                                bt1[:], oh_c, x1s_c[:, 384:768],
                                start=(ci == 0), stop=(ci == nch_g - 1))
                        bt_t = bt_all[:, t * H:(t + 1) * H]
                        if g == 0:
                            nc.vector.tensor_copy(bt_t[:, 0:384], bt0[:])
                            nc.vector.tensor_copy(bt_t[:, 384:768], bt1[:])
                        else:
                            nc.vector.tensor_add(bt_t[:, 0:384],
                                                 bt_t[:, 0:384], bt0[:])
                            nc.vector.tensor_add(bt_t[:, 384:768],
                                                 bt_t[:, 384:768], bt1[:])

                for t in range(NT):
                    l2_tile(0, t)
                for w in range(4):
                    for tt in range(4):
                        l2_tile(1, w * 4 + tt)
                    btT_sb = wp2.tile([128, 6 * 512], BF, tag="btTsb")
                    x1wT_sb = wp2.tile([128, 6 * 512], BF, tag="x1wT")
                    for tt in range(4):
                        t = w * 4 + tt
                        x1o_t = x1own_sb[:, t * H:(t + 1) * H]
                        bt_t = bt_all[:, t * H:(t + 1) * H]
                        for k in range(6):
                            trp = pp2.tile([128, 128], BF, tag="trp",
                                           bufs=2)
                            nc.tensor.transpose(
                                trp[:], bt_t[:, k * 128:(k + 1) * 128],
                                ident_sb[:])
                            nc.scalar.copy(
                                btT_sb[:, k * 512 + tt * 128:
                                       k * 512 + (tt + 1) * 128], trp[:])
                            trq = pp2.tile([128, 128], BF, tag="trp",
                                           bufs=2)
                            nc.tensor.transpose(
                                trq[:], x1o_t[:, k * 128:(k + 1) * 128],
                                ident_sb[:])
                            nc.scalar.copy(
                                x1wT_sb[:, k * 512 + tt * 128:
                                        k * 512 + (tt + 1) * 128], trq[:])

                    for j in range(6):
                        aps = pp2.tile([128, 512], FP32, tag="agg2")
                        first = True
                        for k in range(6):
                            nc.tensor.matmul(
                                aps[:],
                                lint_sb[:, k * H + j * 128:
                                        k * H + (j + 1) * 128],
                                btT_sb[:, k * 512:(k + 1) * 512],
                                start=first, stop=False)
                            first = False
                            nc.tensor.matmul(
                                aps[:],
                                selft_sb[:, k * H + j * 128:
                                         k * H + (j + 1) * 128],
                                x1wT_sb[:, k * 512:(k + 1) * 512],
                                start=False, stop=False)
                        nc.tensor.matmul(
                            aps[:], b2_sb[:, j * 128:(j + 1) * 128],
                            degones_sb[:, w * 512:(w + 1) * 512],
                            start=False, stop=True)
                        x2 = wp2.tile([128, 512], FP32, tag="x2")
                        nc.scalar.activation(x2[:], aps[:], ACTF.Relu)
                        red = wp2.tile([128, 1], FP32, tag="red")
                        nc.vector.reduce_sum(red[:], x2[:], axis=AX.X)
                        nc.vector.tensor_add(pooled_sb[:, j:j + 1],
                                             pooled_sb[:, j:j + 1], red[:])

            nc.sync.dma_start(pooled_d[:], pooled_sb[:])

    nc.compile()
    return nc


def make_in_maps(inputs, sch):
    x = np.asarray(inputs["x"], np.float32)
    relwt = np.ascontiguousarray(
        np.asarray(inputs["rel_W"], np.float32).transpose(0, 2, 1) * 32.0
    ).astype(F8)
    # lint is applied to the fp8-scaled aggregate: fold 1/X1SCALE here.
    lint = np.ascontiguousarray(
        np.asarray(inputs["mp_lin_W"], np.float32).T / X1SCALE).astype(BF16)
    selft = np.ascontiguousarray(
        np.asarray(inputs["mp_self_W"], np.float32).T).astype(BF16)
    b2 = np.stack([np.asarray(inputs["mp_lin_b"], np.float32),
                   np.asarray(inputs["mp_self_b"], np.float32)])
    xbm = x.astype(BF16)
    iota = np.tile(np.arange(128, dtype=np.float32), (128, 1))
    in_maps = []
    for c in range(NCORES):
        cd = sch["cores"][c]
        xown = np.ascontiguousarray(
            x[cd["own"]].reshape(NT, 128, DIN).transpose(1, 0, 2)
            .reshape(128, NT * DIN)).astype(F8)
        in_maps.append(dict(
            xb=xbm, xown=xown, relwt=relwt,
            relb=np.asarray(inputs["rel_b"], np.float32) * 4.0,
            lint=lint, selft=selft, b2=b2, degones=cd["degones"],
            src1=cd["src1"], slot1=cd["slot1"],
            rn1=cd["rn1"], ohuT1=cd["ohuT1"],
            src2=cd["src2"], oh2=cd["oh2"],
            iota=iota, ident=np.eye(128, dtype=BF16)))
    return in_maps


def prep_from_inputs(inputs):
    ei = np.asarray(inputs["edge_index"], np.int64)
    et = np.asarray(inputs["edge_type"], np.int64)
    nrm = np.asarray(inputs["norm_constants"], np.float32)
    invnc = (1.0 / nrm)[et].astype(np.float32)
    return _prep(ei[0], ei[1], et, invnc)


def kernel(**inputs) -> np.ndarray:
    out_W = np.asarray(inputs["out_W"], np.float32)
    out_b = np.asarray(inputs["out_b"], np.float32)

    import time as _t
    t0 = _t.time()
    sch = prep_from_inputs(inputs)
    print(f"[kernel] prep {_t.time()-t0:.1f}s  E1={sch['E1']} E2={sch['E2']}",
          flush=True)
    t0 = _t.time()
    nc = _build(sch)
    print(f"[kernel] build+compile {_t.time()-t0:.1f}s", flush=True)

    in_maps = make_in_maps(inputs, sch)

    t0 = _t.time()
    tmpdir = os.environ.get("TRACE_TMPDIR")
    if tmpdir:
        os.makedirs(tmpdir, exist_ok=True)
    res = bass_utils.run_bass_kernel_spmd(
        nc, in_maps, core_ids=list(range(NCORES)), tmpdir=tmpdir)
    print(f"[kernel] run {_t.time()-t0:.1f}s", flush=True)
    if res.instructions_and_trace is not None:
        print(f"[kernel] trace: {res.instructions_and_trace[1]}", flush=True)
    if res.profile_json is not None:
        print(f"[kernel] profile_json: {res.profile_json}", flush=True)

    pooled = np.zeros(H, np.float64)
    for c in range(NCORES):
        p = res.results[c]["pooled"]  # [128, 6]
        pooled += p.T.reshape(-1).astype(np.float64)  # h = j*128 + p

    kernel._last_exec_ns = res.exec_time_ns

    out = (pooled / N).astype(np.float32) @ out_W.T + out_b
    return out.astype(np.float32)


# revision 20
# speedup vs baseline: 1.3263x; 1.0350x over previous
"""Trainium2 Bass kernel for a 2-layer relational GNN (EvalNet).

Strategy (v2): shard by destination node with a *balanced* node->
(core,tile,slot) assignment (equalizes per-(core,tile,relation) edge
counts, minimizing gather padding). Layer-1 aggregations are core-local.

Layer 1 per dst tile: gather x[src] rows (bf16) per edge; reconstruct
x[dst] per edge ON-CHIP as a one-hot matmul against the tile's own x
(eliminating the dst gather entirely); edge weight via DVE
scalar_tensor_tensor with row-accumulate; weighted one-hot scatter into
per-relation PSUM; apply rel_W after aggregation.

x1 is AllGathered in fp8 (nag=2 slabs, overlapped with L1). Layer 2
gathers fp8 x1 rows per edge (src-deduped per (tile,slab) with
multi-hot one-hots streamed from host), scatters with oh2 as the
stationary matmul operand, transposes the aggregate on-chip, and
applies mp_lin/mp_self after aggregation. The own-node (self) path uses
the locally kept bf16 x1 (no gather). pooled is reduced on-chip;
the final 5-way projection of the 768-d pooled mean is on host.
"""

import os
import sys

sys.path.insert(0, "/opt/trn_rl_repo")

import numpy as np
import ml_dtypes

import concourse.bacc as bacc
import concourse.tile as tile
import concourse.mybir as mybir
from concourse import bass_utils
from concourse.library_config import mlp as mlp_lib

BF16 = ml_dtypes.bfloat16
F8 = ml_dtypes.float8_e4m3

N = 16384
E = 262144
R = 9
DIN = 384
H = 768
NCLS = 5
NCORES = 8
NOWN = N // NCORES          # 2048 nodes per core
NT = NOWN // 128            # 16 dst tiles of 128 per core
NBIN = NCORES * NT          # 128 (core,tile) bins
NAG = 2                     # AllGather slabs
X1SCALE = 0.125             # x1 -> fp8 scale (undone in lint host-side)

FP32 = mybir.dt.float32
BF = mybir.dt.bfloat16
F8E4 = mybir.dt.float8e4
I16 = mybir.dt.int16
AX = mybir.AxisListType
ALU = mybir.AluOpType
ACTF = mybir.ActivationFunctionType


def _wrap16(ids):
    """int16 index layout for dma_gather: [128, n/16], element i at
    [i%16 (+16r replicas), i//16]."""
    a = np.asarray(ids, np.int16).reshape(-1, 16).T  # [16, n/16]
    return np.ascontiguousarray(np.tile(a, (8, 1)))


def _roundup(x, m):
    return (x + m - 1) // m * m


def _balance_bins(dst, et):
    """Assign nodes to NBIN bins of 128 slots, balancing the 9-dim
    in-degree vectors. Returns pos[node] = global position (bin*128+slot)."""
    deg = np.zeros((N, R), np.int64)
    np.add.at(deg, (dst, et), 1)
    order = np.argsort(-deg.sum(1), kind="stable")
    bin_cnt = np.zeros((NBIN, R), np.float64)
    bin_n = np.zeros(NBIN, np.int64)
    pos = np.zeros(N, np.int64)
    for n in order:
        d = deg[n].astype(np.float64)
        # min over open bins of dot(current load, node's degree vector)
        cost = bin_cnt @ d + 1e-6 * bin_cnt.sum(1)
        cost[bin_n >= 128] = np.inf
        b = int(np.argmin(cost))
        pos[n] = b * 128 + bin_n[b]
        bin_n[b] += 1
        bin_cnt[b] += d
    assert (bin_n == 128).all()
    return pos


def _ag_row(pos):
    """x1 table row (after slab-major AllGather) for global position pos."""
    c, loc = pos // NOWN, pos % NOWN
    g, l = loc // (NOWN // NAG), loc % (NOWN // NAG)
    return g * (NCORES * (NOWN // NAG)) + c * (NOWN // NAG) + l


def _prep(src, dst, et, invnc):
    pos = _balance_bins(dst, et)
    dcore = pos[dst] // NOWN
    dtile = (pos[dst] % NOWN) // 128
    dslot = pos[dst] % 128
    # source AG slab (for L2 gather gating): slab of the src node's tile
    sslab = (pos[src] % NOWN) // (NOWN // NAG)

    per_core = [np.nonzero(dcore == c)[0] for c in range(NCORES)]

    # ---------- L1: group by (tile, rel) ----------
    counts1 = np.zeros((NCORES, NT, R), np.int64)
    for c in range(NCORES):
        e = per_core[c]
        np.add.at(counts1[c], (dtile[e], et[e]), 1)
    K1 = _roundup(counts1.max(axis=0), 128)     # [NT, R]
    S1 = K1.sum(axis=1)                          # [NT]
    E1 = int(S1.sum())
    NCH1 = E1 // 128

    sched1 = []
    for t in range(NT):
        gs, c0 = [], 0
        for r in range(R):
            nch = int(K1[t, r]) // 128
            if nch:
                gs.append((r, c0, c0 + nch))
                c0 += nch
        sched1.append(gs)

    # ---------- L2: group by (tile, slab), dedup by src ----------
    # distinct-src counts per (core,tile,slab)
    counts2 = np.zeros((NCORES, NT, NAG), np.int64)
    groups2 = {}
    for c in range(NCORES):
        e = per_core[c]
        for t in range(NT):
            sel_t = e[dtile[e] == t]
            for g in range(NAG):
                es = sel_t[sslab[sel_t] == g]
                srcs, inv_idx = np.unique(src[es], return_inverse=True)
                counts2[c, t, g] = srcs.size
                groups2[(c, t, g)] = (srcs, inv_idx, es)
    K2 = _roundup(np.maximum(counts2.max(axis=0), 1), 128)  # [NT, NAG]
    S2 = K2.sum(axis=1)
    E2 = int(S2.sum())
    NCH2 = E2 // 128

    sched2 = []   # per tile: [(g, chunk0, chunk1)]
    for t in range(NT):
        gs, c0 = [], 0
        for g in range(NAG):
            nch = int(K2[t, g]) // 128
            gs.append((g, c0, c0 + nch))
            c0 += nch
        sched2.append(gs)

    cores = []
    for c in range(NCORES):
        src1 = np.zeros(E1, np.int64)
        slot1 = np.full(E1, -1.0, np.float32)
        inv1 = np.zeros(E1, np.float32)
        rel1 = np.full(E1, -1, np.int64)
        off = 0
        e_all = per_core[c]
        for t in range(NT):
            for r in range(R):
                k = int(K1[t, r])
                if k == 0:
                    continue
                es = e_all[(dtile[e_all] == t) & (et[e_all] == r)]
                n = es.size
                src1[off:off + n] = src[es]
                slot1[off:off + n] = dslot[es]
                inv1[off:off + n] = invnc[es]
                rel1[off:off + n] = r
                off += k
        assert off == E1

        # ohuT1[s, e] one-hot of slot (fp8), rn1[e%128, chunk*R + r]
        ohuT1 = np.zeros((128, E1), F8)
        val = slot1 >= 0
        idx = np.nonzero(val)[0]
        ohuT1[slot1[idx].astype(np.int64), idx] = inv1[idx]
        rn1 = np.zeros((128, NCH1 * R), BF16)
        rn1[idx % 128, (idx // 128) * R + rel1[idx]] = 1.0

        src2 = np.zeros(E2, np.int64)
        oh2 = np.zeros((128, E2), np.float32)
        off = 0
        for t in range(NT):
            for g in range(NAG):
                k = int(K2[t, g])
                srcs, inv_idx, es = groups2[(c, t, g)]
                n = srcs.size
                src2[off:off + n] = _ag_row(pos[srcs])
                np.add.at(oh2, (dslot[es], off + inv_idx), 1.0)
                off += k
        assert off == E2
        oh2 = oh2.astype(F8)
        # reshape oh2 to [128 p=e%128, chunk*128 + s]
        oh2v = np.zeros((128, NCH2 * 128), F8)
        er = np.arange(E2)
        oh2v[:, :] = oh2.T.reshape(NCH2, 128, 128).transpose(1, 0, 2).reshape(128, -1)

        def wrap128(v):
            o = np.zeros((128, v.size // 128), np.float32)
            p = np.arange(v.size)
            o[p % 128, p // 128] = v
            return o

        own = np.argsort(pos)[c * NOWN:(c + 1) * NOWN]  # node ids in (t,s) order
        deg2 = np.zeros(NOWN, np.float32)
        lp = pos[dst[e_all]] - c * NOWN
        np.add.at(deg2, lp, 1.0)
        degones = np.stack([deg2, np.ones(NOWN, np.float32)])

        cores.append(dict(
            src1=_wrap16(src1), slot1=wrap128(slot1),
            rn1=rn1, ohuT1=ohuT1,
            src2=_wrap16(src2), oh2=oh2v,
            degones=degones, own=own,
        ))

    return dict(E1=E1, NCH1=NCH1, S1=S1, sched1=sched1,
                E2=E2, NCH2=NCH2, S2=S2, sched2=sched2,
                K2m=int(K2.max()) // 128, cores=cores, pos=pos)


def _build(sch):
    E1, NCH1, S1, sched1 = sch["E1"], sch["NCH1"], sch["S1"], sch["sched1"]
    E2, NCH2, S2, sched2 = sch["E2"], sch["NCH2"], sch["S2"], sch["sched2"]
    G1 = max(int(s) for s in S1) // 128
    G2 = max(int(s) for s in S2) // 128
    K2m = sch["K2m"]
    gblk = int(os.environ.get("GBLK", 8))
    gblk2 = int(os.environ.get("GBLK2", 8))
    scratch = int(os.environ.get("DMA_SCRATCH", 16384))

    nc = bacc.Bacc("TRN2", target_bir_lowering=False, debug=False,
                   num_devices=NCORES, dynamic_dma_scratch_size=scratch)

    xb_d = nc.dram_tensor("xb", [N, DIN], BF, kind="ExternalInput")
    xown_d = nc.dram_tensor("xown", [128, NT * DIN], F8E4, kind="ExternalInput")
    relwt_d = nc.dram_tensor("relwt", [R, DIN, H], F8E4, kind="ExternalInput")
    relb_d = nc.dram_tensor("relb", [R, H], FP32, kind="ExternalInput")
    lint_d = nc.dram_tensor("lint", [H, H], BF, kind="ExternalInput")
    selft_d = nc.dram_tensor("selft", [H, H], BF, kind="ExternalInput")
    b2_d = nc.dram_tensor("b2", [2, H], FP32, kind="ExternalInput")
    degones_d = nc.dram_tensor("degones", [2, NOWN], FP32, kind="ExternalInput")
    src1_d = nc.dram_tensor("src1", [128, E1 // 16], I16, kind="ExternalInput")
    slot1_d = nc.dram_tensor("slot1", [128, NCH1], FP32, kind="ExternalInput")
    rn1_d = nc.dram_tensor("rn1", [128, NCH1 * R], BF, kind="ExternalInput")
    ohuT1_d = nc.dram_tensor("ohuT1", [128, E1], F8E4, kind="ExternalInput")
    src2_d = nc.dram_tensor("src2", [128, E2 // 16], I16, kind="ExternalInput")
    oh2_d = nc.dram_tensor("oh2", [128, NCH2 * 128], F8E4, kind="ExternalInput")
    iota_d = nc.dram_tensor("iota", [128, 128], FP32, kind="ExternalInput")
    ident_d = nc.dram_tensor("ident", [128, 128], BF, kind="ExternalInput")
    pooled_d = nc.dram_tensor("pooled", [128, 6], FP32, kind="ExternalOutput")

    with tile.TileContext(nc) as tc:
        nc.gpsimd.load_library(mlp_lib)
        with (
            tc.tile_pool(name="const", bufs=1) as cp,
            tc.tile_pool(name="dram", bufs=1, space="DRAM") as dp,
        ):
            # ---- metadata / small constants (gate first gathers) ----
            src1_sb = cp.tile([128, E1 // 16], I16)
            nc.sync.dma_start(src1_sb[:], src1_d[:])
            slot1_sb = cp.tile([128, NCH1], FP32)
            nc.sync.dma_start(slot1_sb[:], slot1_d[:])
            rn1_sb = cp.tile([128, NCH1 * R], BF)
            nc.sync.dma_start(rn1_sb[:], rn1_d[:])
            iota_sb = cp.tile([128, 128], FP32)
            nc.sync.dma_start(iota_sb[:], iota_d[:])
            ident_sb = cp.tile([128, 128], BF)
            nc.sync.dma_start(ident_sb[:], ident_d[:])
            src2_sb = cp.tile([128, E2 // 16], I16)
            nc.sync.dma_start(src2_sb[:], src2_d[:])
            xown_sb = cp.tile([128, NT * DIN], F8E4)
            nc.sync.dma_start(xown_sb[:], xown_d[:])
            relb_sb = cp.tile([R, H], FP32)
            nc.sync.dma_start(relb_sb[:], relb_d[:])
            b2_sb = cp.tile([2, H], FP32)
            nc.sync.dma_start(b2_sb[:], b2_d[:])
            degones_sb = cp.tile([2, NOWN], FP32)
            nc.sync.dma_start(degones_sb[:], degones_d[:])
            lint_sb = cp.tile([128, 6 * H], BF)
            selft_sb = cp.tile([128, 6 * H], BF)
            nc.sync.dma_start(
                lint_sb.rearrange("p (k h) -> p k h", h=H)[:, :, :],
                lint_d.rearrange("(k p) h -> p k h", p=128)[:, :, :])
            nc.sync.dma_start(
                selft_sb.rearrange("p (k h) -> p k h", h=H)[:, :, :],
                selft_d.rearrange("(k p) h -> p k h", p=128)[:, :, :])
            pooled_sb = cp.tile([128, 6], FP32)
            nc.vector.memset(pooled_sb[:], 0.0)
            # x1 kept locally (bf16) + transposed-on-demand for L2 self path
            x1own_sb = cp.tile([128, NT * H], BF)

            cc_in = dp.tile([NOWN, H], F8E4)
            cc_out = dp.tile([N, H], F8E4)

            def sub_gather(dst_tile, src_ap, idx_sb, chunk0, nchunks, elem,
                           blk, out_chunk0=0):
                v3 = dst_tile.rearrange("p (c d) -> p c d", d=elem)
                for b0 in range(0, nchunks, blk):
                    b1 = min(b0 + blk, nchunks)
                    col = (chunk0 + b0) * 8
                    nc.gpsimd.dma_gather(
                        v3[:, out_chunk0 + b0:out_chunk0 + b1, :], src_ap,
                        idx_sb[:, col:col + (b1 - b0) * 8],
                        (b1 - b0) * 128, (b1 - b0) * 128, elem,
                        single_packet=False)

            # ================= Layer 1 =================
            with (
                tc.tile_pool(name="w1c", bufs=1) as wc,
                tc.tile_pool(name="w1", bufs=2) as wp,
                tc.tile_pool(name="ps1", bufs=2, space="PSUM") as pp,
            ):
                relwt_sb = wc.tile([128, R * 3 * H], F8E4)
                nc.sync.dma_start(
                    relwt_sb.rearrange("p (r k h) -> p r k h", k=3, h=H)[:, :, :, :],
                    relwt_d.rearrange("r (k p) h -> p r k h", p=128)[:, :, :, :])

                chunk_base = 0
                for t in range(NT):
                    ncht = int(S1[t]) // 128
                    xs_g = wp.tile([128, G1 * DIN], BF, tag="xs", bufs=4)
                    sub_gather(xs_g, xb_d[:], src1_sb, chunk_base, ncht, DIN,
                               gblk)
                    ohuT_t = wp.tile([128, G1 * 128], F8E4, tag="ohuT", bufs=3)
                    nc.sync.dma_start(
                        ohuT_t[:, :ncht * 128],
                        ohuT1_d[:, chunk_base * 128:(chunk_base + ncht) * 128])

                    art_sb = wp.tile([128, R * 3 * 128], F8E4, tag="artsb")
                    ct_ps = pp.tile([R, 128], FP32, tag="ct", bufs=1)

                    first_ct = True
                    for (r, gc0, gc1) in sched1[t]:
                        art_ps = pp.tile([128, 3 * 128], FP32, tag="art",
                                         bufs=2)
                        for ci in range(gc0, gc1):
                            gci = chunk_base + ci
                            xs_c = xs_g[:, ci * DIN:(ci + 1) * DIN]
                            # reconstruct x_dst rows for this chunk on PE
                            xd_ps = pp.tile([128, DIN], FP32, tag="xd",
                                            bufs=2)
                            nc.tensor.matmul(
                                xd_ps[:],
                                ohuT_t[:, ci * 128:(ci + 1) * 128],
                                xown_sb[:, t * DIN:(t + 1) * DIN],
                                start=True, stop=True)
                            norm = wp.tile([128, 1], FP32, tag="norm", bufs=8)
                            prod = wp.tile([128, DIN], BF, tag="prod", bufs=6)
                            nc.vector.scalar_tensor_tensor(
                                prod[:], xs_c, 1.0, xd_ps[:],
                                ALU.mult, ALU.mult, accum_out=norm[:])
                            ohw = wp.tile([128, 128], BF, tag="ohw", bufs=8)
                            nc.vector.tensor_scalar(
                                ohw[:], iota_sb[:], slot1_sb[:, gci:gci + 1],
                                norm[:], ALU.is_equal, ALU.mult)
                            nc.tensor.matmul(
                                ct_ps[:], rn1_sb[:, gci * R:(gci + 1) * R],
                                ohw[:], start=first_ct,
                                stop=(ci == sched1[t][-1][2] - 1))
                            first_ct = False
                            for k in range(3):
                                nc.tensor.matmul(
                                    art_ps[:, k * 128:(k + 1) * 128],
                                    xs_c[:, k * 128:(k + 1) * 128],
                                    ohw[:], start=(ci == gc0 and k == 0),
                                    stop=(ci == gc1 - 1 and k == 2))
                        nc.scalar.activation(
                            art_sb[:, r * 384:(r + 1) * 384], art_ps[:],
                            ACTF.Copy, scale=0.125)

                    ct_sb = wp.tile([R, 128], FP32, tag="ctsb")
                    nc.vector.tensor_copy(ct_sb[:], ct_ps[:])

                    x1t = wp.tile([128, H], BF, tag="x1t")
                    relwt_v = relwt_sb.rearrange("p (r k h) -> p r k h",
                                                 k=3, h=H)
                    for s in range(2):
                        mps = pp.tile([128, 384], FP32, tag="mps", bufs=2)
                        first = True
                        for (r, _, _) in sched1[t]:
                            nc.tensor.matmul(
                                mps[:],
                                art_sb[:, r * 384:r * 384 + 256].rearrange(
                                    "p (k m) -> p k m", k=2),
                                relwt_v[:, r, 0:2, s * 384:(s + 1) * 384],
                                start=first, stop=False,
                                perf_mode=mybir.MatmulPerfMode.DoubleRow)
                            first = False
                            nc.tensor.matmul(
                                mps[:],
                                art_sb[:, r * 384 + 256:(r + 1) * 384],
                                relwt_v[:, r, 2, s * 384:(s + 1) * 384],
                                start=False, stop=False)
                        nc.tensor.matmul(mps[:], ct_sb[:],
                                         relb_sb[:, s * 384:(s + 1) * 384],
                                         start=False, stop=True)
                        nc.scalar.activation(x1t[:, s * 384:(s + 1) * 384],
                                             mps[:], ACTF.Relu, scale=0.25)
                    nc.vector.tensor_copy(x1own_sb[:, t * H:(t + 1) * H],
                                          x1t[:])
                    x1q = wp.tile([128, H], F8E4, tag="x1q")
                    nc.scalar.activation(x1q[:], x1t[:], ACTF.Copy,
                                         scale=X1SCALE)
                    nc.sync.dma_start(cc_in[t * 128:(t + 1) * 128, :], x1q[:])
                    chunk_base += ncht

                    tper = NT // NAG
                    if (t + 1) % tper == 0:
                        g = (t + 1) // tper - 1
                        rows = NOWN // NAG
                        nc.gpsimd.collective_compute(
                            "AllGather", ALU.bypass,
                            replica_groups=[list(range(NCORES))],
                            ins=[cc_in[g * rows:(g + 1) * rows, :].opt()],
                            outs=[cc_out[g * NCORES * rows:
                                         (g + 1) * NCORES * rows, :].opt()])

            # ================= Layer 2 =================
            with (
                tc.tile_pool(name="w2", bufs=2) as wp2,
                tc.tile_pool(name="ps2", bufs=2, space="PSUM") as pp2,
            ):
                tb2 = np.concatenate([[0], np.cumsum(S2 // 128)]).astype(int)
                bt_all = wp2.tile([128, NT * H], BF, tag="btall", bufs=1)

                # phases A (slab 0) then B (slab 1): the slab-1 gathers wait
                # on the 2nd AllGather; keeping them out of the gpsimd stream
                # until all slab-0 gathers are issued hides that latency.
                def l2_tile(g, t):
                        gsl = [x for x in sched2[t] if x[0] == g]
                        (_, gc0, gc1) = gsl[0]
                        nch_g = gc1 - gc0
                        base = int(tb2[t])
                        x1s_g = wp2.tile([128, K2m * H], F8E4, tag="x1s", bufs=4)
                        oh2_t = wp2.tile([128, K2m * 128], F8E4, tag="oh2t", bufs=4)
                        nc.sync.dma_start(
                            oh2_t[:, :nch_g * 128],
                            oh2_d[:, (base + gc0) * 128:
                                  (base + gc1) * 128])
                        sub_gather(x1s_g, cc_out[:], src2_sb,
                                   base + gc0, nch_g, H, gblk2)
                        bt0 = pp2.tile([128, 384], FP32, tag="btp", bufs=2)
                        bt1 = pp2.tile([128, 384], FP32, tag="btq", bufs=2)
                        for ci in range(nch_g):
                            x1s_c = x1s_g[:, ci * H:(ci + 1) * H]
                            oh_c = oh2_t[:, ci * 128:(ci + 1) * 128]
                            nc.tensor.matmul(
                                bt0[:], oh_c, x1s_c[:, 0:384],
                                start=(ci == 0), stop=(ci == nch_g - 1))
                            nc.tensor.matmul(
                                bt1[:], oh_c, x1s_c[:, 384:768],
                                start=(ci == 0), stop=(ci == nch_g - 1))
                        bt_t = bt_all[:, t * H:(t + 1) * H]
                        if g == 0:
                            nc.vector.tensor_copy(bt_t[:, 0:384], bt0[:])
                            nc.vector.tensor_copy(bt_t[:, 384:768], bt1[:])
                        else:
                            nc.vector.tensor_add(bt_t[:, 0:384],
                                                 bt_t[:, 0:384], bt0[:])
                            nc.vector.tensor_add(bt_t[:, 384:768],
                                                 bt_t[:, 384:768], bt1[:])

                for t in range(NT):
                    l2_tile(0, t)
                for w in range(4):
                    for tt in range(4):
                        l2_tile(1, w * 4 + tt)
                    btT_sb = wp2.tile([128, 6 * 512], BF, tag="btTsb")
                    x1wT_sb = wp2.tile([128, 6 * 512], BF, tag="x1wT")
                    for tt in range(4):
                        t = w * 4 + tt
                        x1o_t = x1own_sb[:, t * H:(t + 1) * H]
                        bt_t = bt_all[:, t * H:(t + 1) * H]
                        for k in range(6):
                            trp = pp2.tile([128, 128], BF, tag="trp",
                                           bufs=2)
                            nc.tensor.transpose(
                                trp[:], bt_t[:, k * 128:(k + 1) * 128],
                                ident_sb[:])
                            nc.scalar.copy(
                                btT_sb[:, k * 512 + tt * 128:
                                       k * 512 + (tt + 1) * 128], trp[:])
                            trq = pp2.tile([128, 128], BF, tag="trp",
                                           bufs=2)
                            nc.tensor.transpose(
                                trq[:], x1o_t[:, k * 128:(k + 1) * 128],
                                ident_sb[:])
                            nc.scalar.copy(
                                x1wT_sb[:, k * 512 + tt * 128:
                                        k * 512 + (tt + 1) * 128], trq[:])

                    for j in range(6):
                        aps = pp2.tile([128, 512], FP32, tag="agg2")
                        first = True
                        for k in range(6):
                            nc.tensor.matmul(
                                aps[:],
                                lint_sb[:, k * H + j * 128:
                                        k * H + (j + 1) * 128],
                                btT_sb[:, k * 512:(k + 1) * 512],
                                start=first, stop=False)
                            first = False
                            nc.tensor.matmul(
                                aps[:],
                                selft_sb[:, k * H + j * 128:
                                         k * H + (j + 1) * 128],
                                x1wT_sb[:, k * 512:(k + 1) * 512],
                                start=False, stop=False)
                        nc.tensor.matmul(
                            aps[:], b2_sb[:, j * 128:(j + 1) * 128],
                            degones_sb[:, w * 512:(w + 1) * 512],
                            start=False, stop=True)
                        x2 = wp2.tile([128, 512], FP32, tag="x2")
                        nc.scalar.activation(x2[:], aps[:], ACTF.Relu)
                        red = wp2.tile([128, 1], FP32, tag="red")
                        nc.vector.reduce_sum(red[:], x2[:], axis=AX.X)
                        nc.vector.tensor_add(pooled_sb[:, j:j + 1],
                                             pooled_sb[:, j:j + 1], red[:])

            nc.sync.dma_start(pooled_d[:], pooled_sb[:])

    nc.compile()
    return nc


def make_in_maps(inputs, sch):
    x = np.asarray(inputs["x"], np.float32)
    relwt = np.ascontiguousarray(
        np.asarray(inputs["rel_W"], np.float32).transpose(0, 2, 1) * 32.0
    ).astype(F8)
    # lint is applied to the fp8-scaled aggregate: fold 1/X1SCALE here.
    lint = np.ascontiguousarray(
        np.asarray(inputs["mp_lin_W"], np.float32).T / X1SCALE).astype(BF16)
    selft = np.ascontiguousarray(
        np.asarray(inputs["mp_self_W"], np.float32).T).astype(BF16)
    b2 = np.stack([np.asarray(inputs["mp_lin_b"], np.float32),
                   np.asarray(inputs["mp_self_b"], np.float32)])
    xbm = x.astype(BF16)
    iota = np.tile(np.arange(128, dtype=np.float32), (128, 1))
    in_maps = []
    for c in range(NCORES):
        cd = sch["cores"][c]
        xown = np.ascontiguousarray(
            x[cd["own"]].reshape(NT, 128, DIN).transpose(1, 0, 2)
            .reshape(128, NT * DIN)).astype(F8)
        in_maps.append(dict(
            xb=xbm, xown=xown, relwt=relwt,
            relb=np.asarray(inputs["rel_b"], np.float32) * 4.0,
            lint=lint, selft=selft, b2=b2, degones=cd["degones"],
            src1=cd["src1"], slot1=cd["slot1"],
            rn1=cd["rn1"], ohuT1=cd["ohuT1"],
            src2=cd["src2"], oh2=cd["oh2"],
            iota=iota, ident=np.eye(128, dtype=BF16)))
    return in_maps


def prep_from_inputs(inputs):
    ei = np.asarray(inputs["edge_index"], np.int64)
    et = np.asarray(inputs["edge_type"], np.int64)
    nrm = np.asarray(inputs["norm_constants"], np.float32)
    invnc = (1.0 / nrm)[et].astype(np.float32)
    return _prep(ei[0], ei[1], et, invnc)


def kernel(**inputs) -> np.ndarray:
    out_W = np.asarray(inputs["out_W"], np.float32)
    out_b = np.asarray(inputs["out_b"], np.float32)

    import time as _t
    t0 = _t.time()
    sch = prep_from_inputs(inputs)
    print(f"[kernel] prep {_t.time()-t0:.1f}s  E1={sch['E1']} E2={sch['E2']}",
          flush=True)
    t0 = _t.time()
    nc = _build(sch)
    print(f"[kernel] build+compile {_t.time()-t0:.1f}s", flush=True)

    in_maps = make_in_maps(inputs, sch)

    t0 = _t.time()
    tmpdir = os.environ.get("TRACE_TMPDIR")
    if tmpdir:
        os.makedirs(tmpdir, exist_ok=True)
    res = bass_utils.run_bass_kernel_spmd(
        nc, in_maps, core_ids=list(range(NCORES)), tmpdir=tmpdir)
    print(f"[kernel] run {_t.time()-t0:.1f}s", flush=True)
    if res.instructions_and_trace is not None:
        print(f"[kernel] trace: {res.instructions_and_trace[1]}", flush=True)
    if res.profile_json is not None:
        print(f"[kernel] profile_json: {res.profile_json}", flush=True)

    pooled = np.zeros(H, np.float64)
    for c in range(NCORES):
        p = res.results[c]["pooled"]  # [128, 6]
        pooled += p.T.reshape(-1).astype(np.float64)  # h = j*128 + p

    kernel._last_exec_ns = res.exec_time_ns

    out = (pooled / N).astype(np.float32) @ out_W.T + out_b
    return out.astype(np.float32)


# revision 22
# speedup vs baseline: 1.3614x; 1.0265x over previous
"""Trainium2 Bass kernel for a 2-layer relational GNN (EvalNet).

Strategy (v2): shard by destination node with a *balanced* node->
(core,tile,slot) assignment (equalizes per-(core,tile,relation) edge
counts, minimizing gather padding). Layer-1 aggregations are core-local.

Layer 1 per dst tile: gather x[src] rows (bf16) per edge; reconstruct
x[dst] per edge ON-CHIP as a one-hot matmul against the tile's own x
(eliminating the dst gather entirely); edge weight via DVE
scalar_tensor_tensor with row-accumulate; weighted one-hot scatter into
per-relation PSUM; apply rel_W after aggregation.

x1 is AllGathered in fp8 (nag=2 slabs, overlapped with L1). Layer 2
gathers fp8 x1 rows per edge (src-deduped per (tile,slab) with
multi-hot one-hots streamed from host), scatters with oh2 as the
stationary matmul operand, transposes the aggregate on-chip, and
applies mp_lin/mp_self after aggregation. The own-node (self) path uses
the locally kept bf16 x1 (no gather). pooled is reduced on-chip;
the final 5-way projection of the 768-d pooled mean is on host.
"""

import os
import sys

sys.path.insert(0, "/opt/trn_rl_repo")

import numpy as np
import ml_dtypes

import concourse.bacc as bacc
import concourse.tile as tile
import concourse.mybir as mybir
from concourse import bass_utils
from concourse.library_config import mlp as mlp_lib

BF16 = ml_dtypes.bfloat16
F8 = ml_dtypes.float8_e4m3

N = 16384
E = 262144
R = 9
DIN = 384
H = 768
NCLS = 5
NCORES = 8
NOWN = N // NCORES          # 2048 nodes per core
NT = NOWN // 128            # 16 dst tiles of 128 per core
NBIN = NCORES * NT          # 128 (core,tile) bins
NAG = 2                     # AllGather slabs
X1SCALE = 0.125             # x1 -> fp8 scale (undone in lint host-side)

FP32 = mybir.dt.float32
BF = mybir.dt.bfloat16
F8E4 = mybir.dt.float8e4
I16 = mybir.dt.int16
AX = mybir.AxisListType
ALU = mybir.AluOpType
ACTF = mybir.ActivationFunctionType


def _wrap16(ids):
    """int16 index layout for dma_gather: [128, n/16], element i at
    [i%16 (+16r replicas), i//16]."""
    a = np.asarray(ids, np.int16).reshape(-1, 16).T  # [16, n/16]
    return np.ascontiguousarray(np.tile(a, (8, 1)))


def _roundup(x, m):
    return (x + m - 1) // m * m


def _balance_bins(dst, et):
    """Assign nodes to NBIN bins of 128 slots, balancing the 9-dim
    in-degree vectors. Returns pos[node] = global position (bin*128+slot)."""
    deg = np.zeros((N, R), np.int64)
    np.add.at(deg, (dst, et), 1)
    order = np.argsort(-deg.sum(1), kind="stable")
    bin_cnt = np.zeros((NBIN, R), np.float64)
    bin_n = np.zeros(NBIN, np.int64)
    pos = np.zeros(N, np.int64)
    for n in order:
        d = deg[n].astype(np.float64)
        # min over open bins of dot(current load, node's degree vector)
        cost = bin_cnt @ d + 1e-6 * bin_cnt.sum(1)
        cost[bin_n >= 128] = np.inf
        b = int(np.argmin(cost))
        pos[n] = b * 128 + bin_n[b]
        bin_n[b] += 1
        bin_cnt[b] += d
    assert (bin_n == 128).all()
    return pos


def _ag_row(pos):
    """x1 table row (after slab-major AllGather) for global position pos."""
    c, loc = pos // NOWN, pos % NOWN
    g, l = loc // (NOWN // NAG), loc % (NOWN // NAG)
    return g * (NCORES * (NOWN // NAG)) + c * (NOWN // NAG) + l


def _prep(src, dst, et, invnc):
    pos = _balance_bins(dst, et)
    dcore = pos[dst] // NOWN
    dtile = (pos[dst] % NOWN) // 128
    dslot = pos[dst] % 128
    # source AG slab (for L2 gather gating): slab of the src node's tile
    sslab = (pos[src] % NOWN) // (NOWN // NAG)

    per_core = [np.nonzero(dcore == c)[0] for c in range(NCORES)]

    # ---------- L1: group by (tile, rel) ----------
    counts1 = np.zeros((NCORES, NT, R), np.int64)
    for c in range(NCORES):
        e = per_core[c]
        np.add.at(counts1[c], (dtile[e], et[e]), 1)
    K1 = _roundup(counts1.max(axis=0), 128)     # [NT, R]
    S1 = K1.sum(axis=1)                          # [NT]
    E1 = int(S1.sum())
    NCH1 = E1 // 128

    sched1 = []
    for t in range(NT):
        gs, c0 = [], 0
        for r in range(R):
            nch = int(K1[t, r]) // 128
            if nch:
                gs.append((r, c0, c0 + nch))
                c0 += nch
        sched1.append(gs)

    # ---------- L2: group by (tile, slab), dedup by src ----------
    # distinct-src counts per (core,tile,slab)
    counts2 = np.zeros((NCORES, NT, NAG), np.int64)
    groups2 = {}
    for c in range(NCORES):
        e = per_core[c]
        for t in range(NT):
            sel_t = e[dtile[e] == t]
            for g in range(NAG):
                es = sel_t[sslab[sel_t] == g]
                srcs, inv_idx = np.unique(src[es], return_inverse=True)
                counts2[c, t, g] = srcs.size
                groups2[(c, t, g)] = (srcs, inv_idx, es)
    K2 = _roundup(np.maximum(counts2.max(axis=0), 1), 128)  # [NT, NAG]
    S2 = K2.sum(axis=1)
    E2 = int(S2.sum())
    NCH2 = E2 // 128

    sched2 = []   # per tile: [(g, chunk0, chunk1)]
    for t in range(NT):
        gs, c0 = [], 0
        for g in range(NAG):
            nch = int(K2[t, g]) // 128
            gs.append((g, c0, c0 + nch))
            c0 += nch
        sched2.append(gs)

    cores = []
    for c in range(NCORES):
        src1 = np.zeros(E1, np.int64)
        slot1 = np.full(E1, -1.0, np.float32)
        inv1 = np.zeros(E1, np.float32)
        rel1 = np.full(E1, -1, np.int64)
        off = 0
        e_all = per_core[c]
        for t in range(NT):
            for r in range(R):
                k = int(K1[t, r])
                if k == 0:
                    continue
                es = e_all[(dtile[e_all] == t) & (et[e_all] == r)]
                n = es.size
                src1[off:off + n] = src[es]
                slot1[off:off + n] = dslot[es]
                inv1[off:off + n] = invnc[es]
                rel1[off:off + n] = r
                off += k
        assert off == E1

        # ohuT1[s, e] one-hot of slot (fp8), rn1[e%128, chunk*R + r]
        ohuT1 = np.zeros((128, E1), F8)
        val = slot1 >= 0
        idx = np.nonzero(val)[0]
        ohuT1[slot1[idx].astype(np.int64), idx] = inv1[idx]
        rn1 = np.zeros((128, NCH1 * R), BF16)
        rn1[idx % 128, (idx // 128) * R + rel1[idx]] = 1.0

        src2 = np.zeros(E2, np.int64)
        oh2 = np.zeros((128, E2), np.float32)
        off = 0
        for t in range(NT):
            for g in range(NAG):
                k = int(K2[t, g])
                srcs, inv_idx, es = groups2[(c, t, g)]
                n = srcs.size
                src2[off:off + n] = _ag_row(pos[srcs]) - g * (N // NAG)
                np.add.at(oh2, (dslot[es], off + inv_idx), 1.0)
                off += k
        assert off == E2
        oh2 = oh2.astype(F8)
        # reshape oh2 to [128 p=e%128, chunk*128 + s]
        oh2v = np.zeros((128, NCH2 * 128), F8)
        er = np.arange(E2)
        oh2v[:, :] = oh2.T.reshape(NCH2, 128, 128).transpose(1, 0, 2).reshape(128, -1)

        def wrap128(v):
            o = np.zeros((128, v.size // 128), np.float32)
            p = np.arange(v.size)
            o[p % 128, p // 128] = v
            return o

        own = np.argsort(pos)[c * NOWN:(c + 1) * NOWN]  # node ids in (t,s) order
        deg2 = np.zeros(NOWN, np.float32)
        lp = pos[dst[e_all]] - c * NOWN
        np.add.at(deg2, lp, 1.0)
        degones = np.stack([deg2, np.ones(NOWN, np.float32)])

        cores.append(dict(
            src1=_wrap16(src1), slot1=wrap128(slot1),
            rn1=rn1, ohuT1=ohuT1,
            src2=_wrap16(src2), oh2=oh2v,
            degones=degones, own=own,
        ))

    return dict(E1=E1, NCH1=NCH1, S1=S1, sched1=sched1,
                E2=E2, NCH2=NCH2, S2=S2, sched2=sched2,
                K2m=int(K2.max()) // 128, cores=cores, pos=pos)


def _build(sch):
    E1, NCH1, S1, sched1 = sch["E1"], sch["NCH1"], sch["S1"], sch["sched1"]
    E2, NCH2, S2, sched2 = sch["E2"], sch["NCH2"], sch["S2"], sch["sched2"]
    G1 = max(int(s) for s in S1) // 128
    G2 = max(int(s) for s in S2) // 128
    K2m = sch["K2m"]
    gblk = int(os.environ.get("GBLK", 9))
    gblk2 = int(os.environ.get("GBLK2", 8))
    scratch = int(os.environ.get("DMA_SCRATCH", 32768))

    nc = bacc.Bacc("TRN2", target_bir_lowering=False, debug=False,
                   num_devices=NCORES, dynamic_dma_scratch_size=scratch)

    xb_d = nc.dram_tensor("xb", [N, DIN], BF, kind="ExternalInput")
    xown_d = nc.dram_tensor("xown", [128, NT * DIN], F8E4, kind="ExternalInput")
    relwt_d = nc.dram_tensor("relwt", [R, DIN, H], F8E4, kind="ExternalInput")
    relb_d = nc.dram_tensor("relb", [R, H], FP32, kind="ExternalInput")
    lint_d = nc.dram_tensor("lint", [H, H], BF, kind="ExternalInput")
    selft_d = nc.dram_tensor("selft", [H, H], BF, kind="ExternalInput")
    b2_d = nc.dram_tensor("b2", [2, H], FP32, kind="ExternalInput")
    degones_d = nc.dram_tensor("degones", [2, NOWN], FP32, kind="ExternalInput")
    src1_d = nc.dram_tensor("src1", [128, E1 // 16], I16, kind="ExternalInput")
    slot1_d = nc.dram_tensor("slot1", [128, NCH1], FP32, kind="ExternalInput")
    rn1_d = nc.dram_tensor("rn1", [128, NCH1 * R], BF, kind="ExternalInput")
    ohuT1_d = nc.dram_tensor("ohuT1", [128, E1], F8E4, kind="ExternalInput")
    src2_d = nc.dram_tensor("src2", [128, E2 // 16], I16, kind="ExternalInput")
    oh2_d = nc.dram_tensor("oh2", [128, NCH2 * 128], F8E4, kind="ExternalInput")
    iota_d = nc.dram_tensor("iota", [128, 128], FP32, kind="ExternalInput")
    ident_d = nc.dram_tensor("ident", [128, 128], BF, kind="ExternalInput")
    pooled_d = nc.dram_tensor("pooled", [128, 6], FP32, kind="ExternalOutput")

    with tile.TileContext(nc) as tc:
        nc.gpsimd.load_library(mlp_lib)
        with (
            tc.tile_pool(name="const", bufs=1) as cp,
            tc.tile_pool(name="dram", bufs=1, space="DRAM") as dp,
        ):
            # ---- metadata / small constants (gate first gathers) ----
            src1_sb = cp.tile([128, E1 // 16], I16)
            nc.sync.dma_start(src1_sb[:], src1_d[:])
            slot1_sb = cp.tile([128, NCH1], FP32)
            nc.sync.dma_start(slot1_sb[:], slot1_d[:])
            rn1_sb = cp.tile([128, NCH1 * R], BF)
            nc.sync.dma_start(rn1_sb[:], rn1_d[:])
            iota_sb = cp.tile([128, 128], FP32)
            nc.sync.dma_start(iota_sb[:], iota_d[:])
            ident_sb = cp.tile([128, 128], BF)
            nc.sync.dma_start(ident_sb[:], ident_d[:])
            src2_sb = cp.tile([128, E2 // 16], I16)
            nc.sync.dma_start(src2_sb[:], src2_d[:])
            xown_sb = cp.tile([128, NT * DIN], F8E4)
            nc.sync.dma_start(xown_sb[:], xown_d[:])
            relb_sb = cp.tile([R, H], FP32)
            nc.sync.dma_start(relb_sb[:], relb_d[:])
            b2_sb = cp.tile([2, H], FP32)
            nc.sync.dma_start(b2_sb[:], b2_d[:])
            degones_sb = cp.tile([2, NOWN], FP32)
            nc.sync.dma_start(degones_sb[:], degones_d[:])
            lint_sb = cp.tile([128, 6 * H], BF)
            selft_sb = cp.tile([128, 6 * H], BF)
            nc.sync.dma_start(
                lint_sb.rearrange("p (k h) -> p k h", h=H)[:, :, :],
                lint_d.rearrange("(k p) h -> p k h", p=128)[:, :, :])
            nc.sync.dma_start(
                selft_sb.rearrange("p (k h) -> p k h", h=H)[:, :, :],
                selft_d.rearrange("(k p) h -> p k h", p=128)[:, :, :])
            pooled_sb = cp.tile([128, 6], FP32)
            nc.vector.memset(pooled_sb[:], 0.0)
            # x1 kept locally (bf16) + transposed-on-demand for L2 self path
            x1own_sb = cp.tile([128, NT * H], BF)

            cc_in = dp.tile([NOWN, H], F8E4)
            cc_out = [dp.tile([N // NAG, H], F8E4, name=f"cc_out{_g}")
                      for _g in range(NAG)]

            def sub_gather(dst_tile, src_ap, idx_sb, chunk0, nchunks, elem,
                           blk, out_chunk0=0):
                v3 = dst_tile.rearrange("p (c d) -> p c d", d=elem)
                for b0 in range(0, nchunks, blk):
                    b1 = min(b0 + blk, nchunks)
                    col = (chunk0 + b0) * 8
                    nc.gpsimd.dma_gather(
                        v3[:, out_chunk0 + b0:out_chunk0 + b1, :], src_ap,
                        idx_sb[:, col:col + (b1 - b0) * 8],
                        (b1 - b0) * 128, (b1 - b0) * 128, elem,
                        single_packet=False)

            # ================= Layer 1 =================
            with (
                tc.tile_pool(name="w1c", bufs=1) as wc,
                tc.tile_pool(name="w1", bufs=2) as wp,
                tc.tile_pool(name="ps1", bufs=2, space="PSUM") as pp,
            ):
                relwt_sb = wc.tile([128, R * 3 * H], F8E4)
                nc.sync.dma_start(
                    relwt_sb.rearrange("p (r k h) -> p r k h", k=3, h=H)[:, :, :, :],
                    relwt_d.rearrange("r (k p) h -> p r k h", p=128)[:, :, :, :])

                chunk_base = 0
                for t in range(NT):
                    ncht = int(S1[t]) // 128
                    xs_g = wp.tile([128, G1 * DIN], BF, tag="xs", bufs=3)
                    sub_gather(xs_g, xb_d[:], src1_sb, chunk_base, ncht, DIN,
                               gblk)
                    ohuT_t = wp.tile([128, G1 * 128], F8E4, tag="ohuT", bufs=3)
                    nc.sync.dma_start(
                        ohuT_t[:, :ncht * 128],
                        ohuT1_d[:, chunk_base * 128:(chunk_base + ncht) * 128])

                    art_sb = wp.tile([128, R * 3 * 128], F8E4, tag="artsb")
                    ct_ps = pp.tile([R, 128], FP32, tag="ct", bufs=1)

                    first_ct = True
                    for (r, gc0, gc1) in sched1[t]:
                        art_ps = pp.tile([128, 3 * 128], FP32, tag="art",
                                         bufs=2)
                        for ci in range(gc0, gc1):
                            gci = chunk_base + ci
                            xs_c = xs_g[:, ci * DIN:(ci + 1) * DIN]
                            # reconstruct x_dst rows for this chunk on PE
                            xd_ps = pp.tile([128, DIN], FP32, tag="xd",
                                            bufs=2)
                            nc.tensor.matmul(
                                xd_ps[:],
                                ohuT_t[:, ci * 128:(ci + 1) * 128],
                                xown_sb[:, t * DIN:(t + 1) * DIN],
                                start=True, stop=True)
                            norm = wp.tile([128, 1], FP32, tag="norm", bufs=8)
                            prod = wp.tile([128, DIN], BF, tag="prod", bufs=6)
                            nc.vector.scalar_tensor_tensor(
                                prod[:], xs_c, 1.0, xd_ps[:],
                                ALU.mult, ALU.mult, accum_out=norm[:])
                            ohw = wp.tile([128, 128], BF, tag="ohw", bufs=8)
                            nc.vector.tensor_scalar(
                                ohw[:], iota_sb[:], slot1_sb[:, gci:gci + 1],
                                norm[:], ALU.is_equal, ALU.mult)
                            nc.tensor.matmul(
                                ct_ps[:], rn1_sb[:, gci * R:(gci + 1) * R],
                                ohw[:], start=first_ct,
                                stop=(ci == sched1[t][-1][2] - 1))
                            first_ct = False
                            for k in range(3):
                                nc.tensor.matmul(
                                    art_ps[:, k * 128:(k + 1) * 128],
                                    xs_c[:, k * 128:(k + 1) * 128],
                                    ohw[:], start=(ci == gc0 and k == 0),
                                    stop=(ci == gc1 - 1 and k == 2))
                        nc.scalar.activation(
                            art_sb[:, r * 384:(r + 1) * 384], art_ps[:],
                            ACTF.Copy, scale=0.125)

                    ct_sb = wp.tile([R, 128], FP32, tag="ctsb")
                    nc.vector.tensor_copy(ct_sb[:], ct_ps[:])

                    x1t = wp.tile([128, H], BF, tag="x1t")
                    relwt_v = relwt_sb.rearrange("p (r k h) -> p r k h",
                                                 k=3, h=H)
                    for s in range(2):
                        mps = pp.tile([128, 384], FP32, tag="mps", bufs=2)
                        first = True
                        for (r, _, _) in sched1[t]:
                            nc.tensor.matmul(
                                mps[:],
                                art_sb[:, r * 384:r * 384 + 256].rearrange(
                                    "p (k m) -> p k m", k=2),
                                relwt_v[:, r, 0:2, s * 384:(s + 1) * 384],
                                start=first, stop=False,
                                perf_mode=mybir.MatmulPerfMode.DoubleRow)
                            first = False
                            nc.tensor.matmul(
                                mps[:],
                                art_sb[:, r * 384 + 256:(r + 1) * 384],
                                relwt_v[:, r, 2, s * 384:(s + 1) * 384],
                                start=False, stop=False)
                        nc.tensor.matmul(mps[:], ct_sb[:],
                                         relb_sb[:, s * 384:(s + 1) * 384],
                                         start=False, stop=True)
                        nc.scalar.activation(x1t[:, s * 384:(s + 1) * 384],
                                             mps[:], ACTF.Relu, scale=0.25)
                    nc.vector.tensor_copy(x1own_sb[:, t * H:(t + 1) * H],
                                          x1t[:])
                    x1q = wp.tile([128, H], F8E4, tag="x1q")
                    nc.scalar.activation(x1q[:], x1t[:], ACTF.Copy,
                                         scale=X1SCALE)
                    nc.sync.dma_start(cc_in[t * 128:(t + 1) * 128, :], x1q[:])
                    chunk_base += ncht

                    tper = NT // NAG
                    if (t + 1) % tper == 0:
                        g = (t + 1) // tper - 1
                        rows = NOWN // NAG
                        nc.gpsimd.collective_compute(
                            "AllGather", ALU.bypass,
                            replica_groups=[list(range(NCORES))],
                            ins=[cc_in[g * rows:(g + 1) * rows, :].opt()],
                            outs=[cc_out[g][:, :].opt()])

            # ================= Layer 2 =================
            with (
                tc.tile_pool(name="w2", bufs=2) as wp2,
                tc.tile_pool(name="ps2", bufs=2, space="PSUM") as pp2,
            ):
                tb2 = np.concatenate([[0], np.cumsum(S2 // 128)]).astype(int)
                bt_all = wp2.tile([128, NT * H], BF, tag="btall", bufs=1)

                # phases A (slab 0) then B (slab 1): the slab-1 gathers wait
                # on the 2nd AllGather; keeping them out of the gpsimd stream
                # until all slab-0 gathers are issued hides that latency.
                def l2_tile(g, t):
                        gsl = [x for x in sched2[t] if x[0] == g]
                        (_, gc0, gc1) = gsl[0]
                        nch_g = gc1 - gc0
                        base = int(tb2[t])
                        x1s_g = wp2.tile([128, K2m * H], F8E4, tag="x1s", bufs=4)
                        oh2_t = wp2.tile([128, K2m * 128], F8E4, tag="oh2t", bufs=4)
                        nc.sync.dma_start(
                            oh2_t[:, :nch_g * 128],
                            oh2_d[:, (base + gc0) * 128:
                                  (base + gc1) * 128])
                        sub_gather(x1s_g, cc_out[g][:], src2_sb,
                                   base + gc0, nch_g, H, gblk2)
                        bt0 = pp2.tile([128, 384], FP32, tag="btp", bufs=2)
                        bt1 = pp2.tile([128, 384], FP32, tag="btq", bufs=2)
                        for ci in range(nch_g):
                            x1s_c = x1s_g[:, ci * H:(ci + 1) * H]
                            oh_c = oh2_t[:, ci * 128:(ci + 1) * 128]
                            nc.tensor.matmul(
                                bt0[:], oh_c, x1s_c[:, 0:384],
                                start=(ci == 0), stop=(ci == nch_g - 1))
                            nc.tensor.matmul(
                                bt1[:], oh_c, x1s_c[:, 384:768],
                                start=(ci == 0), stop=(ci == nch_g - 1))
                        bt_t = bt_all[:, t * H:(t + 1) * H]
                        if g == 0:
                            nc.vector.tensor_copy(bt_t[:, 0:384], bt0[:])
                            nc.vector.tensor_copy(bt_t[:, 384:768], bt1[:])
                        else:
                            nc.vector.tensor_add(bt_t[:, 0:384],
                                                 bt_t[:, 0:384], bt0[:])
                            nc.vector.tensor_add(bt_t[:, 384:768],
                                                 bt_t[:, 384:768], bt1[:])

                for t in range(NT):
                    l2_tile(0, t)
                for w in range(4):
                    for tt in range(4):
                        l2_tile(1, w * 4 + tt)
                    btT_sb = wp2.tile([128, 6 * 512], BF, tag="btTsb")
                    x1wT_sb = wp2.tile([128, 6 * 512], BF, tag="x1wT")
                    for tt in range(4):
                        t = w * 4 + tt
                        x1o_t = x1own_sb[:, t * H:(t + 1) * H]
                        bt_t = bt_all[:, t * H:(t + 1) * H]
                        for k in range(6):
                            trp = pp2.tile([128, 128], BF, tag="trp",
                                           bufs=2)
                            nc.tensor.transpose(
                                trp[:], bt_t[:, k * 128:(k + 1) * 128],
                                ident_sb[:])
                            nc.scalar.copy(
                                btT_sb[:, k * 512 + tt * 128:
                                       k * 512 + (tt + 1) * 128], trp[:])
                            trq = pp2.tile([128, 128], BF, tag="trp",
                                           bufs=2)
                            nc.tensor.transpose(
                                trq[:], x1o_t[:, k * 128:(k + 1) * 128],
                                ident_sb[:])
                            nc.scalar.copy(
                                x1wT_sb[:, k * 512 + tt * 128:
                                        k * 512 + (tt + 1) * 128], trq[:])

                    for j in range(6):
                        aps = pp2.tile([128, 512], FP32, tag="agg2")
                        first = True
                        for k in range(6):
                            nc.tensor.matmul(
                                aps[:],
                                lint_sb[:, k * H + j * 128:
                                        k * H + (j + 1) * 128],
                                btT_sb[:, k * 512:(k + 1) * 512],
                                start=first, stop=False)
                            first = False
                            nc.tensor.matmul(
                                aps[:],
                                selft_sb[:, k * H + j * 128:
                                         k * H + (j + 1) * 128],
                                x1wT_sb[:, k * 512:(k + 1) * 512],
                                start=False, stop=False)
                        nc.tensor.matmul(
                            aps[:], b2_sb[:, j * 128:(j + 1) * 128],
                            degones_sb[:, w * 512:(w + 1) * 512],
                            start=False, stop=True)
                        x2 = wp2.tile([128, 512], FP32, tag="x2")
                        nc.scalar.activation(x2[:], aps[:], ACTF.Relu)
                        red = wp2.tile([128, 1], FP32, tag="red")
                        nc.vector.reduce_sum(red[:], x2[:], axis=AX.X)
                        nc.vector.tensor_add(pooled_sb[:, j:j + 1],
                                             pooled_sb[:, j:j + 1], red[:])

            nc.sync.dma_start(pooled_d[:], pooled_sb[:])

    nc.compile()
    return nc


def make_in_maps(inputs, sch):
    x = np.asarray(inputs["x"], np.float32)
    relwt = np.ascontiguousarray(
        np.asarray(inputs["rel_W"], np.float32).transpose(0, 2, 1) * 32.0
    ).astype(F8)
    # lint is applied to the fp8-scaled aggregate: fold 1/X1SCALE here.
    lint = np.ascontiguousarray(
        np.asarray(inputs["mp_lin_W"], np.float32).T / X1SCALE).astype(BF16)
    selft = np.ascontiguousarray(
        np.asarray(inputs["mp_self_W"], np.float32).T).astype(BF16)
    b2 = np.stack([np.asarray(inputs["mp_lin_b"], np.float32),
                   np.asarray(inputs["mp_self_b"], np.float32)])
    xbm = x.astype(BF16)
    iota = np.tile(np.arange(128, dtype=np.float32), (128, 1))
    in_maps = []
    for c in range(NCORES):
        cd = sch["cores"][c]
        xown = np.ascontiguousarray(
            x[cd["own"]].reshape(NT, 128, DIN).transpose(1, 0, 2)
            .reshape(128, NT * DIN)).astype(F8)
        in_maps.append(dict(
            xb=xbm, xown=xown, relwt=relwt,
            relb=np.asarray(inputs["rel_b"], np.float32) * 4.0,
            lint=lint, selft=selft, b2=b2, degones=cd["degones"],
            src1=cd["src1"], slot1=cd["slot1"],
            rn1=cd["rn1"], ohuT1=cd["ohuT1"],
            src2=cd["src2"], oh2=cd["oh2"],
            iota=iota, ident=np.eye(128, dtype=BF16)))
    return in_maps


def prep_from_inputs(inputs):
    ei = np.asarray(inputs["edge_index"], np.int64)
    et = np.asarray(inputs["edge_type"], np.int64)
    nrm = np.asarray(inputs["norm_constants"], np.float32)
    invnc = (1.0 / nrm)[et].astype(np.float32)
    return _prep(ei[0], ei[1], et, invnc)


def kernel(**inputs) -> np.ndarray:
    out_W = np.asarray(inputs["out_W"], np.float32)
    out_b = np.asarray(inputs["out_b"], np.float32)

    import time as _t
    t0 = _t.time()
    sch = prep_from_inputs(inputs)
    print(f"[kernel] prep {_t.time()-t0:.1f}s  E1={sch['E1']} E2={sch['E2']}",
          flush=True)
    t0 = _t.time()
    nc = _build(sch)
    print(f"[kernel] build+compile {_t.time()-t0:.1f}s", flush=True)

    in_maps = make_in_maps(inputs, sch)

    t0 = _t.time()
    tmpdir = os.environ.get("TRACE_TMPDIR")
    if tmpdir:
        os.makedirs(tmpdir, exist_ok=True)
    res = bass_utils.run_bass_kernel_spmd(
        nc, in_maps, core_ids=list(range(NCORES)), tmpdir=tmpdir)
    print(f"[kernel] run {_t.time()-t0:.1f}s", flush=True)
    if res.instructions_and_trace is not None:
        print(f"[kernel] trace: {res.instructions_and_trace[1]}", flush=True)
    if res.profile_json is not None:
        print(f"[kernel] profile_json: {res.profile_json}", flush=True)

    pooled = np.zeros(H, np.float64)
    for c in range(NCORES):
        p = res.results[c]["pooled"]  # [128, 6]
        pooled += p.T.reshape(-1).astype(np.float64)  # h = j*128 + p

    kernel._last_exec_ns = res.exec_time_ns

    out = (pooled / N).astype(np.float32) @ out_W.T + out_b
    return out.astype(np.float32)


# revision 23
# speedup vs baseline: 1.4203x; 1.0433x over previous
"""Trainium2 Bass kernel for a 2-layer relational GNN (EvalNet).

Strategy (v2): shard by destination node with a *balanced* node->
(core,tile,slot) assignment (equalizes per-(core,tile,relation) edge
counts, minimizing gather padding). Layer-1 aggregations are core-local.

Layer 1 per dst tile: gather x[src] rows (bf16) per edge; reconstruct
x[dst] per edge ON-CHIP as a one-hot matmul against the tile's own x
(eliminating the dst gather entirely); edge weight via DVE
scalar_tensor_tensor with row-accumulate; weighted one-hot scatter into
per-relation PSUM; apply rel_W after aggregation.

x1 is AllGathered in fp8 (nag=2 slabs, overlapped with L1). Layer 2
gathers fp8 x1 rows per edge (src-deduped per (tile,slab) with
multi-hot one-hots streamed from host), scatters with oh2 as the
stationary matmul operand, transposes the aggregate on-chip, and
applies mp_lin/mp_self after aggregation. The own-node (self) path uses
the locally kept bf16 x1 (no gather). pooled is reduced on-chip;
the final 5-way projection of the 768-d pooled mean is on host.
"""

import os
import sys

sys.path.insert(0, "/opt/trn_rl_repo")

import numpy as np
import ml_dtypes

import concourse.bacc as bacc
import concourse.tile as tile
import concourse.mybir as mybir
from concourse import bass_utils
from concourse.library_config import mlp as mlp_lib

BF16 = ml_dtypes.bfloat16
F8 = ml_dtypes.float8_e4m3

N = 16384
E = 262144
R = 9
DIN = 384
H = 768
NCLS = 5
NCORES = 8
NOWN = N // NCORES          # 2048 nodes per core
NT = NOWN // 128            # 16 dst tiles of 128 per core
NBIN = NCORES * NT          # 128 (core,tile) bins
NAG = 2                     # AllGather slabs
X1SCALE = 0.125             # x1 -> fp8 scale (undone in lint host-side)

FP32 = mybir.dt.float32
BF = mybir.dt.bfloat16
F8E4 = mybir.dt.float8e4
I16 = mybir.dt.int16
AX = mybir.AxisListType
ALU = mybir.AluOpType
ACTF = mybir.ActivationFunctionType


def _wrap16(ids):
    """int16 index layout for dma_gather: [128, n/16], element i at
    [i%16 (+16r replicas), i//16]."""
    a = np.asarray(ids, np.int16).reshape(-1, 16).T  # [16, n/16]
    return np.ascontiguousarray(np.tile(a, (8, 1)))


def _roundup(x, m):
    return (x + m - 1) // m * m


def _balance_bins(dst, et):
    """Assign nodes to NBIN bins of 128 slots, balancing the 9-dim
    in-degree vectors. Returns pos[node] = global position (bin*128+slot)."""
    deg = np.zeros((N, R), np.int64)
    np.add.at(deg, (dst, et), 1)
    order = np.argsort(-deg.sum(1), kind="stable")
    bin_cnt = np.zeros((NBIN, R), np.float64)
    bin_n = np.zeros(NBIN, np.int64)
    pos = np.zeros(N, np.int64)
    for n in order:
        d = deg[n].astype(np.float64)
        # min over open bins of dot(current load, node's degree vector)
        cost = bin_cnt @ d + 1e-6 * bin_cnt.sum(1)
        cost[bin_n >= 128] = np.inf
        b = int(np.argmin(cost))
        pos[n] = b * 128 + bin_n[b]
        bin_n[b] += 1
        bin_cnt[b] += d
    assert (bin_n == 128).all()
    return pos


def _ag_row(pos):
    """x1 table row (after slab-major AllGather) for global position pos."""
    c, loc = pos // NOWN, pos % NOWN
    g, l = loc // (NOWN // NAG), loc % (NOWN // NAG)
    return g * (NCORES * (NOWN // NAG)) + c * (NOWN // NAG) + l


def _prep(src, dst, et, invnc):
    pos = _balance_bins(dst, et)
    dcore = pos[dst] // NOWN
    dtile = (pos[dst] % NOWN) // 128
    dslot = pos[dst] % 128
    # source AG slab (for L2 gather gating): slab of the src node's tile
    sslab = (pos[src] % NOWN) // (NOWN // NAG)

    per_core = [np.nonzero(dcore == c)[0] for c in range(NCORES)]

    # ---------- L1: group by (tile, rel) ----------
    counts1 = np.zeros((NCORES, NT, R), np.int64)
    for c in range(NCORES):
        e = per_core[c]
        np.add.at(counts1[c], (dtile[e], et[e]), 1)
    K1 = _roundup(counts1.max(axis=0), 128)     # [NT, R]
    S1 = K1.sum(axis=1)                          # [NT]
    E1 = int(S1.sum())
    NCH1 = E1 // 128

    sched1 = []
    for t in range(NT):
        gs, c0 = [], 0
        for r in range(R):
            nch = int(K1[t, r]) // 128
            if nch:
                gs.append((r, c0, c0 + nch))
                c0 += nch
        sched1.append(gs)

    # ---------- L2: group by (tile, slab), dedup by src ----------
    # distinct-src counts per (core,tile,slab)
    counts2 = np.zeros((NCORES, NT, NAG), np.int64)
    groups2 = {}
    for c in range(NCORES):
        e = per_core[c]
        for t in range(NT):
            sel_t = e[dtile[e] == t]
            for g in range(NAG):
                es = sel_t[sslab[sel_t] == g]
                srcs, inv_idx = np.unique(src[es], return_inverse=True)
                counts2[c, t, g] = srcs.size
                groups2[(c, t, g)] = (srcs, inv_idx, es)
    K2 = _roundup(np.maximum(counts2.max(axis=0), 1), 128)  # [NT, NAG]
    S2 = K2.sum(axis=1)
    E2 = int(S2.sum())
    NCH2 = E2 // 128

    sched2 = []   # per tile: [(g, chunk0, chunk1)]
    for t in range(NT):
        gs, c0 = [], 0
        for g in range(NAG):
            nch = int(K2[t, g]) // 128
            gs.append((g, c0, c0 + nch))
            c0 += nch
        sched2.append(gs)

    cores = []
    for c in range(NCORES):
        src1 = np.zeros(E1, np.int64)
        slot1 = np.full(E1, -1.0, np.float32)
        inv1 = np.zeros(E1, np.float32)
        rel1 = np.full(E1, -1, np.int64)
        off = 0
        e_all = per_core[c]
        for t in range(NT):
            for r in range(R):
                k = int(K1[t, r])
                if k == 0:
                    continue
                es = e_all[(dtile[e_all] == t) & (et[e_all] == r)]
                n = es.size
                src1[off:off + n] = src[es]
                slot1[off:off + n] = dslot[es]
                inv1[off:off + n] = invnc[es]
                rel1[off:off + n] = r
                off += k
        assert off == E1

        # ohuT1[s, e] one-hot of slot (fp8), rn1[e%128, chunk*R + r]
        ohuT1 = np.zeros((128, E1), F8)
        val = slot1 >= 0
        idx = np.nonzero(val)[0]
        ohuT1[slot1[idx].astype(np.int64), idx] = inv1[idx]
        rn1 = np.zeros((128, NCH1 * R), BF16)
        rn1[idx % 128, (idx // 128) * R + rel1[idx]] = 1.0

        src2 = np.zeros(E2, np.int64)
        oh2 = np.zeros((128, E2), np.float32)
        off = 0
        for t in range(NT):
            for g in range(NAG):
                k = int(K2[t, g])
                srcs, inv_idx, es = groups2[(c, t, g)]
                n = srcs.size
                src2[off:off + n] = _ag_row(pos[srcs]) - g * (N // NAG)
                np.add.at(oh2, (dslot[es], off + inv_idx), 1.0)
                off += k
        assert off == E2
        oh2 = oh2.astype(F8)
        # reshape oh2 to [128 p=e%128, chunk*128 + s]
        oh2v = np.zeros((128, NCH2 * 128), F8)
        er = np.arange(E2)
        oh2v[:, :] = oh2.T.reshape(NCH2, 128, 128).transpose(1, 0, 2).reshape(128, -1)

        def wrap128(v):
            o = np.zeros((128, v.size // 128), np.float32)
            p = np.arange(v.size)
            o[p % 128, p // 128] = v
            return o

        own = np.argsort(pos)[c * NOWN:(c + 1) * NOWN]  # node ids in (t,s) order
        deg2 = np.zeros(NOWN, np.float32)
        lp = pos[dst[e_all]] - c * NOWN
        np.add.at(deg2, lp, 1.0)
        degones = np.stack([deg2, np.ones(NOWN, np.float32)])

        cores.append(dict(
            src1=_wrap16(src1), slot1=wrap128(slot1),
            rn1=rn1, ohuT1=ohuT1,
            src2=_wrap16(src2), oh2=oh2v,
            degones=degones, own=own,
        ))

    return dict(E1=E1, NCH1=NCH1, S1=S1, sched1=sched1,
                E2=E2, NCH2=NCH2, S2=S2, sched2=sched2,
                K2m=int(K2.max()) // 128, cores=cores, pos=pos)


def _build(sch):
    E1, NCH1, S1, sched1 = sch["E1"], sch["NCH1"], sch["S1"], sch["sched1"]
    E2, NCH2, S2, sched2 = sch["E2"], sch["NCH2"], sch["S2"], sch["sched2"]
    G1 = max(int(s) for s in S1) // 128
    G2 = max(int(s) for s in S2) // 128
    K2m = sch["K2m"]
    gblk = int(os.environ.get("GBLK", 8))
    gblk2 = int(os.environ.get("GBLK2", 8))
    scratch = int(os.environ.get("DMA_SCRATCH", 16384))

    nc = bacc.Bacc("TRN2", target_bir_lowering=False, debug=False,
                   num_devices=NCORES, dynamic_dma_scratch_size=scratch)

    xb_d = nc.dram_tensor("xb", [N, DIN], BF, kind="ExternalInput")
    xown_d = nc.dram_tensor("xown", [128, NT * DIN], F8E4, kind="ExternalInput")
    relwt_d = nc.dram_tensor("relwt", [R, DIN, H], F8E4, kind="ExternalInput")
    relb_d = nc.dram_tensor("relb", [R, H], FP32, kind="ExternalInput")
    lint_d = nc.dram_tensor("lint", [H, H], BF, kind="ExternalInput")
    selft_d = nc.dram_tensor("selft", [H, H], BF, kind="ExternalInput")
    b2_d = nc.dram_tensor("b2", [2, H], FP32, kind="ExternalInput")
    degones_d = nc.dram_tensor("degones", [2, NOWN], FP32, kind="ExternalInput")
    src1_d = nc.dram_tensor("src1", [128, E1 // 16], I16, kind="ExternalInput")
    slot1_d = nc.dram_tensor("slot1", [128, NCH1], FP32, kind="ExternalInput")
    rn1_d = nc.dram_tensor("rn1", [128, NCH1 * R], BF, kind="ExternalInput")
    ohuT1_d = nc.dram_tensor("ohuT1", [128, E1], F8E4, kind="ExternalInput")
    src2_d = nc.dram_tensor("src2", [128, E2 // 16], I16, kind="ExternalInput")
    oh2_d = nc.dram_tensor("oh2", [128, NCH2 * 128], F8E4, kind="ExternalInput")
    iota_d = nc.dram_tensor("iota", [128, 128], FP32, kind="ExternalInput")
    ident_d = nc.dram_tensor("ident", [128, 128], BF, kind="ExternalInput")
    pooled_d = nc.dram_tensor("pooled", [128, 6], FP32, kind="ExternalOutput")

    with tile.TileContext(nc) as tc:
        nc.gpsimd.load_library(mlp_lib)
        with (
            tc.tile_pool(name="const", bufs=1) as cp,
            tc.tile_pool(name="dram", bufs=1, space="DRAM") as dp,
        ):
            # ---- metadata / small constants (gate first gathers) ----
            src1_sb = cp.tile([128, E1 // 16], I16)
            nc.sync.dma_start(src1_sb[:], src1_d[:])
            slot1_sb = cp.tile([128, NCH1], FP32)
            nc.sync.dma_start(slot1_sb[:], slot1_d[:])
            rn1_sb = cp.tile([128, NCH1 * R], BF)
            nc.sync.dma_start(rn1_sb[:], rn1_d[:])
            iota_sb = cp.tile([128, 128], FP32)
            nc.sync.dma_start(iota_sb[:], iota_d[:])
            ident_sb = cp.tile([128, 128], BF)
            nc.sync.dma_start(ident_sb[:], ident_d[:])
            src2_sb = cp.tile([128, E2 // 16], I16)
            nc.sync.dma_start(src2_sb[:], src2_d[:])
            xown_sb = cp.tile([128, NT * DIN], F8E4)
            nc.sync.dma_start(xown_sb[:], xown_d[:])
            relb_sb = cp.tile([R, H], FP32)
            nc.sync.dma_start(relb_sb[:], relb_d[:])
            b2_sb = cp.tile([2, H], FP32)
            nc.sync.dma_start(b2_sb[:], b2_d[:])
            degones_sb = cp.tile([2, NOWN], FP32)
            nc.sync.dma_start(degones_sb[:], degones_d[:])
            lint_sb = cp.tile([128, 6 * H], BF)
            selft_sb = cp.tile([128, 6 * H], BF)
            nc.sync.dma_start(
                lint_sb.rearrange("p (k h) -> p k h", h=H)[:, :, :],
                lint_d.rearrange("(k p) h -> p k h", p=128)[:, :, :])
            nc.sync.dma_start(
                selft_sb.rearrange("p (k h) -> p k h", h=H)[:, :, :],
                selft_d.rearrange("(k p) h -> p k h", p=128)[:, :, :])
            pooled_sb = cp.tile([128, 6], FP32)
            nc.vector.memset(pooled_sb[:], 0.0)
            # x1 kept locally (bf16) + transposed-on-demand for L2 self path
            x1own_sb = cp.tile([128, NT * H], BF)

            cc_in = dp.tile([NOWN, H], F8E4)
            cc_out = [dp.tile([N // NAG, H], F8E4, name=f"cc_out{_g}")
                      for _g in range(NAG)]

            def sub_gather(dst_tile, src_ap, idx_sb, chunk0, nchunks, elem,
                           blk, out_chunk0=0):
                v3 = dst_tile.rearrange("p (c d) -> p c d", d=elem)
                for b0 in range(0, nchunks, blk):
                    b1 = min(b0 + blk, nchunks)
                    col = (chunk0 + b0) * 8
                    nc.gpsimd.dma_gather(
                        v3[:, out_chunk0 + b0:out_chunk0 + b1, :], src_ap,
                        idx_sb[:, col:col + (b1 - b0) * 8],
                        (b1 - b0) * 128, (b1 - b0) * 128, elem,
                        single_packet=False)

            # ================= Layer 1 =================
            with (
                tc.tile_pool(name="w1c", bufs=1) as wc,
                tc.tile_pool(name="w1", bufs=2) as wp,
                tc.tile_pool(name="ps1", bufs=2, space="PSUM") as pp,
            ):
                relwt_sb = wc.tile([128, R * 3 * H], F8E4)
                nc.sync.dma_start(
                    relwt_sb.rearrange("p (r k h) -> p r k h", k=3, h=H)[:, :, :, :],
                    relwt_d.rearrange("r (k p) h -> p r k h", p=128)[:, :, :, :])

                chunk_base = 0
                for t in range(NT):
                    ncht = int(S1[t]) // 128
                    xs_g = wp.tile([128, G1 * DIN], BF, tag="xs", bufs=4)
                    sub_gather(xs_g, xb_d[:], src1_sb, chunk_base, ncht, DIN,
                               gblk)
                    ohuT_t = wp.tile([128, G1 * 128], F8E4, tag="ohuT", bufs=3)
                    nc.sync.dma_start(
                        ohuT_t[:, :ncht * 128],
                        ohuT1_d[:, chunk_base * 128:(chunk_base + ncht) * 128])

                    art_sb = wp.tile([128, R * 3 * 128], F8E4, tag="artsb")
                    ct_ps = pp.tile([R, 128], FP32, tag="ct", bufs=1)

                    first_ct = True
                    for (r, gc0, gc1) in sched1[t]:
                        art_ps = pp.tile([128, 3 * 128], FP32, tag="art",
                                         bufs=2)
                        for ci in range(gc0, gc1):
                            gci = chunk_base + ci
                            xs_c = xs_g[:, ci * DIN:(ci + 1) * DIN]
                            # reconstruct x_dst rows for this chunk on PE
                            xd_ps = pp.tile([128, DIN], FP32, tag="xd",
                                            bufs=2)
                            nc.tensor.matmul(
                                xd_ps[:],
                                ohuT_t[:, ci * 128:(ci + 1) * 128],
                                xown_sb[:, t * DIN:(t + 1) * DIN],
                                start=True, stop=True)
                            norm = wp.tile([128, 1], FP32, tag="norm", bufs=8)
                            prod = wp.tile([128, DIN], BF, tag="prod", bufs=6)
                            nc.vector.scalar_tensor_tensor(
                                prod[:], xs_c, 1.0, xd_ps[:],
                                ALU.mult, ALU.mult, accum_out=norm[:])
                            ohw = wp.tile([128, 128], BF, tag="ohw", bufs=8)
                            nc.vector.tensor_scalar(
                                ohw[:], iota_sb[:], slot1_sb[:, gci:gci + 1],
                                norm[:], ALU.is_equal, ALU.mult)
                            nc.tensor.matmul(
                                ct_ps[:], rn1_sb[:, gci * R:(gci + 1) * R],
                                ohw[:], start=first_ct,
                                stop=(ci == sched1[t][-1][2] - 1))
                            first_ct = False
                            for k in range(3):
                                nc.tensor.matmul(
                                    art_ps[:, k * 128:(k + 1) * 128],
                                    xs_c[:, k * 128:(k + 1) * 128],
                                    ohw[:], start=(ci == gc0 and k == 0),
                                    stop=(ci == gc1 - 1 and k == 2))
                        nc.scalar.activation(
                            art_sb[:, r * 384:(r + 1) * 384], art_ps[:],
                            ACTF.Copy, scale=0.125)

                    ct_sb = wp.tile([R, 128], FP32, tag="ctsb")
                    nc.vector.tensor_copy(ct_sb[:], ct_ps[:])

                    x1t = wp.tile([128, H], BF, tag="x1t")
                    relwt_v = relwt_sb.rearrange("p (r k h) -> p r k h",
                                                 k=3, h=H)
                    for s in range(2):
                        mps = pp.tile([128, 384], FP32, tag="mps", bufs=2)
                        first = True
                        for (r, _, _) in sched1[t]:
                            nc.tensor.matmul(
                                mps[:],
                                art_sb[:, r * 384:r * 384 + 256].rearrange(
                                    "p (k m) -> p k m", k=2),
                                relwt_v[:, r, 0:2, s * 384:(s + 1) * 384],
                                start=first, stop=False,
                                perf_mode=mybir.MatmulPerfMode.DoubleRow)
                            first = False
                            nc.tensor.matmul(
                                mps[:],
                                art_sb[:, r * 384 + 256:(r + 1) * 384],
                                relwt_v[:, r, 2, s * 384:(s + 1) * 384],
                                start=False, stop=False)
                        nc.tensor.matmul(mps[:], ct_sb[:],
                                         relb_sb[:, s * 384:(s + 1) * 384],
                                         start=False, stop=True)
                        nc.scalar.activation(x1t[:, s * 384:(s + 1) * 384],
                                             mps[:], ACTF.Relu, scale=0.25)
                    nc.vector.tensor_copy(x1own_sb[:, t * H:(t + 1) * H],
                                          x1t[:])
                    x1q = wp.tile([128, H], F8E4, tag="x1q")
                    nc.scalar.activation(x1q[:], x1t[:], ACTF.Copy,
                                         scale=X1SCALE)
                    nc.sync.dma_start(cc_in[t * 128:(t + 1) * 128, :], x1q[:])
                    chunk_base += ncht

                    tper = NT // NAG
                    if (t + 1) % tper == 0:
                        g = (t + 1) // tper - 1
                        rows = NOWN // NAG
                        nc.gpsimd.collective_compute(
                            "AllGather", ALU.bypass,
                            replica_groups=[list(range(NCORES))],
                            ins=[cc_in[g * rows:(g + 1) * rows, :].opt()],
                            outs=[cc_out[g][:, :].opt()])

            # ================= Layer 2 =================
            with (
                tc.tile_pool(name="w2", bufs=2) as wp2,
                tc.tile_pool(name="ps2", bufs=2, space="PSUM") as pp2,
            ):
                tb2 = np.concatenate([[0], np.cumsum(S2 // 128)]).astype(int)
                bt_all = wp2.tile([128, NT * H], BF, tag="btall", bufs=1)

                # phases A (slab 0) then B (slab 1): the slab-1 gathers wait
                # on the 2nd AllGather; keeping them out of the gpsimd stream
                # until all slab-0 gathers are issued hides that latency.
                def l2_tile(g, t):
                        gsl = [x for x in sched2[t] if x[0] == g]
                        (_, gc0, gc1) = gsl[0]
                        nch_g = gc1 - gc0
                        base = int(tb2[t])
                        x1s_g = wp2.tile([128, K2m * H], F8E4, tag="x1s", bufs=4)
                        oh2_t = wp2.tile([128, K2m * 128], F8E4, tag="oh2t", bufs=4)
                        nc.sync.dma_start(
                            oh2_t[:, :nch_g * 128],
                            oh2_d[:, (base + gc0) * 128:
                                  (base + gc1) * 128])
                        sub_gather(x1s_g, cc_out[g][:], src2_sb,
                                   base + gc0, nch_g, H, gblk2)
                        bt0 = pp2.tile([128, 384], FP32, tag="btp", bufs=2)
                        bt1 = pp2.tile([128, 384], FP32, tag="btq", bufs=2)
                        for ci in range(nch_g):
                            x1s_c = x1s_g[:, ci * H:(ci + 1) * H]
                            oh_c = oh2_t[:, ci * 128:(ci + 1) * 128]
                            nc.tensor.matmul(
                                bt0[:], oh_c, x1s_c[:, 0:384],
                                start=(ci == 0), stop=(ci == nch_g - 1))
                            nc.tensor.matmul(
                                bt1[:], oh_c, x1s_c[:, 384:768],
                                start=(ci == 0), stop=(ci == nch_g - 1))
                        bt_t = bt_all[:, t * H:(t + 1) * H]
                        if g == 0:
                            nc.vector.tensor_copy(bt_t[:, 0:384], bt0[:])
                            nc.vector.tensor_copy(bt_t[:, 384:768], bt1[:])
                        else:
                            nc.vector.tensor_add(bt_t[:, 0:384],
                                                 bt_t[:, 0:384], bt0[:])
                            nc.vector.tensor_add(bt_t[:, 384:768],
                                                 bt_t[:, 384:768], bt1[:])

                for t in range(NT):
                    l2_tile(0, t)
                for w in range(4):
                    for tt in range(4):
                        l2_tile(1, w * 4 + tt)
                    btT_sb = wp2.tile([128, 6 * 512], BF, tag="btTsb")
                    x1wT_sb = wp2.tile([128, 6 * 512], BF, tag="x1wT")
                    for tt in range(4):
                        t = w * 4 + tt
                        x1o_t = x1own_sb[:, t * H:(t + 1) * H]
                        bt_t = bt_all[:, t * H:(t + 1) * H]
                        for k in range(6):
                            trp = pp2.tile([128, 128], BF, tag="trp",
                                           bufs=2)
                            nc.tensor.transpose(
                                trp[:], bt_t[:, k * 128:(k + 1) * 128],
                                ident_sb[:])
                            nc.scalar.copy(
                                btT_sb[:, k * 512 + tt * 128:
                                       k * 512 + (tt + 1) * 128], trp[:])
                            trq = pp2.tile([128, 128], BF, tag="trp",
                                           bufs=2)
                            nc.tensor.transpose(
                                trq[:], x1o_t[:, k * 128:(k + 1) * 128],
                                ident_sb[:])
                            nc.scalar.copy(
                                x1wT_sb[:, k * 512 + tt * 128:
                                        k * 512 + (tt + 1) * 128], trq[:])

                    for j in range(6):
                        aps = pp2.tile([128, 512], FP32, tag="agg2")
                        first = True
                        for k in range(6):
                            nc.tensor.matmul(
                                aps[:],
                                lint_sb[:, k * H + j * 128:
                                        k * H + (j + 1) * 128],
                                btT_sb[:, k * 512:(k + 1) * 512],
                                start=first, stop=False)
                            first = False
                            nc.tensor.matmul(
                                aps[:],
                                selft_sb[:, k * H + j * 128:
                                         k * H + (j + 1) * 128],
                                x1wT_sb[:, k * 512:(k + 1) * 512],
                                start=False, stop=False)
                        nc.tensor.matmul(
                            aps[:], b2_sb[:, j * 128:(j + 1) * 128],
                            degones_sb[:, w * 512:(w + 1) * 512],
                            start=False, stop=True)
                        x2 = wp2.tile([128, 512], FP32, tag="x2")
                        nc.scalar.activation(x2[:], aps[:], ACTF.Relu)
                        red = wp2.tile([128, 1], FP32, tag="red")
                        nc.vector.reduce_sum(red[:], x2[:], axis=AX.X)
                        nc.vector.tensor_add(pooled_sb[:, j:j + 1],
                                             pooled_sb[:, j:j + 1], red[:])

            nc.sync.dma_start(pooled_d[:], pooled_sb[:])

    nc.compile()
    return nc


def make_in_maps(inputs, sch):
    x = np.asarray(inputs["x"], np.float32)
    relwt = np.ascontiguousarray(
        np.asarray(inputs["rel_W"], np.float32).transpose(0, 2, 1) * 32.0
    ).astype(F8)
    # lint is applied to the fp8-scaled aggregate: fold 1/X1SCALE here.
    lint = np.ascontiguousarray(
        np.asarray(inputs["mp_lin_W"], np.float32).T / X1SCALE).astype(BF16)
    selft = np.ascontiguousarray(
        np.asarray(inputs["mp_self_W"], np.float32).T).astype(BF16)
    b2 = np.stack([np.asarray(inputs["mp_lin_b"], np.float32),
                   np.asarray(inputs["mp_self_b"], np.float32)])
    xbm = x.astype(BF16)
    iota = np.tile(np.arange(128, dtype=np.float32), (128, 1))
    in_maps = []
    for c in range(NCORES):
        cd = sch["cores"][c]
        xown = np.ascontiguousarray(
            x[cd["own"]].reshape(NT, 128, DIN).transpose(1, 0, 2)
            .reshape(128, NT * DIN)).astype(F8)
        in_maps.append(dict(
            xb=xbm, xown=xown, relwt=relwt,
            relb=np.asarray(inputs["rel_b"], np.float32) * 4.0,
            lint=lint, selft=selft, b2=b2, degones=cd["degones"],
            src1=cd["src1"], slot1=cd["slot1"],
            rn1=cd["rn1"], ohuT1=cd["ohuT1"],
            src2=cd["src2"], oh2=cd["oh2"],
            iota=iota, ident=np.eye(128, dtype=BF16)))
    return in_maps


def prep_from_inputs(inputs):
    ei = np.asarray(inputs["edge_index"], np.int64)
    et = np.asarray(inputs["edge_type"], np.int64)
    nrm = np.asarray(inputs["norm_constants"], np.float32)
    invnc = (1.0 / nrm)[et].astype(np.float32)
    return _prep(ei[0], ei[1], et, invnc)


def kernel(**inputs) -> np.ndarray:
    out_W = np.asarray(inputs["out_W"], np.float32)
    out_b = np.asarray(inputs["out_b"], np.float32)

    import time as _t
    t0 = _t.time()
    sch = prep_from_inputs(inputs)
    print(f"[kernel] prep {_t.time()-t0:.1f}s  E1={sch['E1']} E2={sch['E2']}",
          flush=True)
    t0 = _t.time()
    nc = _build(sch)
    print(f"[kernel] build+compile {_t.time()-t0:.1f}s", flush=True)

    in_maps = make_in_maps(inputs, sch)

    t0 = _t.time()
    tmpdir = os.environ.get("TRACE_TMPDIR")
    if tmpdir:
        os.makedirs(tmpdir, exist_ok=True)
    res = bass_utils.run_bass_kernel_spmd(
        nc, in_maps, core_ids=list(range(NCORES)), tmpdir=tmpdir)
    print(f"[kernel] run {_t.time()-t0:.1f}s", flush=True)
    if res.instructions_and_trace is not None:
        print(f"[kernel] trace: {res.instructions_and_trace[1]}", flush=True)
    if res.profile_json is not None:
        print(f"[kernel] profile_json: {res.profile_json}", flush=True)

    pooled = np.zeros(H, np.float64)
    for c in range(NCORES):
        p = res.results[c]["pooled"]  # [128, 6]
        pooled += p.T.reshape(-1).astype(np.float64)  # h = j*128 + p

    kernel._last_exec_ns = res.exec_time_ns

    out = (pooled / N).astype(np.float32) @ out_W.T + out_b
    return out.astype(np.float32)
